# revision 1
# baseline (speedup 1.0000x reference)
"""DeepseekV4 indexer kernel for 8 trn2 NeuronCores (Bass/Tile).

Strategy (token-sharded, two bass launches):
  - Tokens are split into 16 tiles of 128; core i owns tiles (i, 15-i) so the
    causally-pruned top-k work is balanced across cores.
  - Launch 1 (per core): fused W_fused/Wproj GEMM over the core's 264-token
    halo'd shard -> compressor softmax -> RMSNorm -> RoPE -> compressed K for
    its 64 compressed positions, plus per-token head weights.  Outputs are
    tiny ([64,128] K + [256,64] wts per core).
  - Host: concatenates the per-core K shards into the full [512,128] K
    (the "all-gather"; collectives don't load on this runtime).
  - Launch 2 (per core): q = qr @ Wq.T (fp32), RoPE, qk against full K,
    relu * wts accumulation over 64 heads, causal mask, and iterated
    top-8 (max / max_index / match_replace) producing the top-256 indices
    in descending-score order.  All GEMMs run in exact fp32 so the ordering
    matches the fp32 reference up to fp32 rounding noise.

kernel(**inputs) takes the FULL unsharded inputs and returns [2048,256] int32.
"""
import sys
sys.path.insert(0, '/opt/trn_rl_repo')

from contextlib import ExitStack

import numpy as np

import concourse.bass as bass
import concourse.bacc as bacc
import concourse.tile as tile
from concourse import mybir
from concourse.bass_utils import run_bass_kernel_spmd
from concourse.masks import make_identity

T, HID, QR_DIM, H, D, TOPK, R = 2048, 7168, 1536, 64, 128, 256, 4
C = T // R
NC = 8
EPS = 1e-6
F32 = mybir.dt.float32
I32 = mybir.dt.int32
U32 = mybir.dt.uint32
WTS_SCALE = float(H ** -0.5) * float(D ** -0.5)  # folds q's D**-0.5 into wts
NEG = -1e30

PAIRS = [(i, 15 - i) for i in range(NC)]  # token tiles owned by core i

_cache = {}


# --------------------------------------------------------------------------
# launch 1: compressor -> per-core compressed K (64 rows) + head weights
# --------------------------------------------------------------------------
def _build_l1():
    nc = bacc.Bacc()
    hidden = nc.declare_dram_parameter("hidden", [264, HID], F32, isOutput=False)
    wcomb = nc.declare_dram_parameter("wcomb", [HID, 576], F32, isOutput=False)
    ape = nc.declare_dram_parameter("ape", [8, D], F32, isOutput=False)
    rmsw = nc.declare_dram_parameter("rmsw", [D], F32, isOutput=False)
    cs_k = nc.declare_dram_parameter("cs_k", [64, D], F32, isOutput=False)
    haloflag = nc.declare_dram_parameter("haloflag", [2], F32, isOutput=False)
    k_loc = nc.declare_dram_parameter("k_loc", [64, D], F32, isOutput=True)
    wts_own = nc.declare_dram_parameter("wts_own", [256, H], F32, isOutput=True)

    with tile.TileContext(nc) as tc, ExitStack() as ctx:
        const = ctx.enter_context(tc.tile_pool(name="const", bufs=1))
        big = ctx.enter_context(tc.tile_pool(name="big", bufs=1))
        work = ctx.enter_context(tc.tile_pool(name="work", bufs=2))

        ident = const.tile([128, 128], F32)
        make_identity(nc, ident)

        def tp(ps_out, in_sb):
            p = in_sb.shape[0]
            nc.tensor.transpose(ps_out, in_sb, ident[:p, :p])

        # ---- hiddenT [128, 56, 264] via PE transposes ----
        hidT = big.tile([128, 56, 264], F32)
        with tc.tile_pool(name="stg", bufs=2) as stg, \
             tc.tile_pool(name="tpsA", bufs=2, space="PSUM") as tpsA:
            for (t0, rows) in [(0, 128), (128, 128), (256, 8)]:
                stage = stg.tile([128, HID], F32, tag="stage")
                nc.sync.dma_start(out=stage[:rows, :], in_=hidden[t0:t0 + rows, :])
                for kg in range(14):
                    ps = tpsA.tile([128, 512], F32, tag="tp")
                    for u in range(4):
                        kc = kg * 4 + u
                        tp(ps[:, u * 128:u * 128 + rows],
                           stage[:rows, kc * 128:(kc + 1) * 128])
                    sv = ps.rearrange("p (u x) -> p u x", x=128)[:, :, :rows]
                    nc.scalar.copy(hidT[:, kg * 4:kg * 4 + 4, t0:t0 + rows], sv)

        # ---- fused GEMM: kv_scoreT [4x128, 264] + wtsT [64, 264] ----
        kvt = []
        wts_sb = work.tile([64, 264], F32, tag="wts_sb")
        with tc.tile_pool(name="wstg", bufs=3) as wstg, \
             tc.tile_pool(name="gps", bufs=1, space="PSUM") as gps:
            kvps = [gps.tile([128, 264], F32, tag=f"kvps{m}", name=f"kvps{m}") for m in range(4)]
            wtsps = gps.tile([64, 264], F32, tag="wtsps")
            for kc in range(56):
                wt = wstg.tile([128, 576], F32, tag="wcomb")
                nc.sync.dma_start(out=wt, in_=wcomb[kc * 128:(kc + 1) * 128, :])
                for m in range(4):
                    nc.tensor.matmul(kvps[m], wt[:, m * 128:(m + 1) * 128],
                                     hidT[:, kc, :], start=(kc == 0),
                                     stop=(kc == 55))
                nc.tensor.matmul(wtsps, wt[:, 512:576], hidT[:, kc, :],
                                 start=(kc == 0), stop=(kc == 55))
            for m in range(4):
                t = work.tile([128, 264], F32, tag=f"kvt{m}")
                nc.scalar.copy(t, kvps[m])
                kvt.append(t)
            nc.scalar.mul(wts_sb, wtsps, WTS_SCALE)
        kv_old, kv_new, sc_old, sc_new = kvt

        with tc.tile_pool(name="tpsB", bufs=2, space="PSUM") as tpsB:
            # wts -> [t, h] and out
            for s in range(2):
                ps = tpsB.tile([128, 64], F32, tag="wtp")
                tp(ps, wts_sb[:, 4 + 132 * s:132 + 132 * s])
                ob = work.tile([128, 64], F32, tag="wob")
                nc.scalar.copy(ob, ps)
                nc.sync.dma_start(out=wts_own[128 * s:128 * (s + 1), :], in_=ob)

            # ape transposed + replicated [128, 32, 8]
            ape_st = work.tile([8, D], F32, tag="ape_st")
            nc.sync.dma_start(out=ape_st, in_=ape[:])
            aps = tpsB.tile([128, 8], F32, tag="apetp")
            tp(aps, ape_st)
            apeT = const.tile([128, 8], F32)
            nc.scalar.copy(apeT, aps)
            ape_rep = const.tile([128, 32, 8], F32)
            for g in range(32):
                nc.vector.tensor_copy(ape_rep[:, g, :], apeT)

            # rms weight replicated [32, 128]
            rms_rep = const.tile([32, D], F32)
            nc.sync.dma_start(out=rms_rep, in_=bass.AP(
                tensor=rmsw, offset=0, ap=[[0, 32], [1, D]]))

            cs_st = []
            for s in range(2):
                cst = const.tile([32, D], F32, tag=f"cs{s}", name=f"cs{s}")
                nc.sync.dma_start(out=cst, in_=cs_k[32 * s:32 * s + 32, :])
                cs_st.append(cst)

            hf = []
            for s in range(2):
                h = const.tile([128, 1], F32, tag=f"hf{s}")
                nc.sync.dma_start(out=h, in_=bass.AP(
                    tensor=haloflag, offset=s, ap=[[0, 128], [1, 1]]))
                hf.append(h)

            for s in range(2):
                o = 132 * s
                gates = work.tile([128, 32, 8], F32, tag="gates")
                so_v = sc_old[:, o:o + 128].rearrange("p (g x) -> p g x", x=4)
                sn_v = sc_new[:, o + 4:o + 132].rearrange("p (g x) -> p g x", x=4)
                ko_v = kv_old[:, o:o + 128].rearrange("p (g x) -> p g x", x=4)
                kn_v = kv_new[:, o + 4:o + 132].rearrange("p (g x) -> p g x", x=4)
                nc.vector.tensor_add(gates[:, :, 0:4], so_v, ape_rep[:, :, 0:4])
                nc.vector.tensor_add(gates[:, :, 4:8], sn_v, ape_rep[:, :, 4:8])
                # first group's old slots += -1e30 when strip starts at t=0
                nc.vector.tensor_scalar(gates[:, 0, 0:4], gates[:, 0, 0:4],
                                        hf[s], None, op0=mybir.AluOpType.add)
                gmax = work.tile([128, 32], F32, tag="gmax")
                nc.vector.reduce_max(gmax, gates, axis=mybir.AxisListType.X)
                nc.vector.tensor_sub(gates, gates,
                                     gmax.to_broadcast([128, 32, 8]))
                ex = work.tile([128, 32, 8], F32, tag="ex")
                nc.scalar.activation(ex, gates, mybir.ActivationFunctionType.Exp)
                den = work.tile([128, 32], F32, tag="den")
                nc.vector.reduce_sum(den, ex, axis=mybir.AxisListType.X)
                rec = work.tile([128, 32], F32, tag="rec")
                nc.vector.reciprocal(rec, den)
                w8 = work.tile([128, 32, 8], F32, tag="w8")
                nc.vector.tensor_mul(w8, ex, rec.to_broadcast([128, 32, 8]))
                prod = work.tile([128, 32, 8], F32, tag="prod")
                nc.vector.tensor_mul(prod[:, :, 0:4], w8[:, :, 0:4], ko_v)
                nc.vector.tensor_mul(prod[:, :, 4:8], w8[:, :, 4:8], kn_v)
                comp = work.tile([128, 32], F32, tag="comp")
                nc.vector.reduce_sum(comp, prod, axis=mybir.AxisListType.X)

                cps = tpsB.tile([32, 128], F32, tag="ctp")
                tp(cps, comp)
                compT = work.tile([32, D], F32, tag="compT")
                nc.scalar.copy(compT, cps)

                # RMSNorm over d
                sq = work.tile([32, D], F32, tag="sq")
                nc.vector.tensor_mul(sq, compT, compT)
                ssum = work.tile([32, 1], F32, tag="ssum")
                nc.vector.reduce_sum(ssum, sq, axis=mybir.AxisListType.X)
                nc.vector.tensor_scalar(ssum, ssum, 1.0 / D, EPS,
                                        op0=mybir.AluOpType.mult,
                                        op1=mybir.AluOpType.add)
                rt = work.tile([32, 1], F32, tag="rt")
                nc.scalar.sqrt(rt, ssum)
                rs = work.tile([32, 1], F32, tag="rs")
                nc.vector.reciprocal(rs, rt)
                nc.vector.tensor_scalar(compT, compT, rs, None,
                                        op0=mybir.AluOpType.mult)
                nc.vector.tensor_mul(compT, compT, rms_rep)

                # RoPE at compressed positions (all tiles at base partition 0)
                co = cs_st[s][:, 0:64]
                si = cs_st[s][:, 64:128]
                x1 = compT[:, 0:64]
                x2 = compT[:, 64:128]
                tmp = work.tile([32, D], F32, tag="ktmp")
                kx = work.tile([32, D], F32, tag="kx")
                nc.vector.tensor_mul(kx[:, 0:64], x1, co)
                nc.vector.tensor_mul(tmp[:, 0:64], x2, si)
                nc.vector.tensor_sub(kx[:, 0:64], kx[:, 0:64], tmp[:, 0:64])
                nc.vector.tensor_mul(kx[:, 64:128], x2, co)
                nc.vector.tensor_mul(tmp[:, 64:128], x1, si)
                nc.vector.tensor_add(kx[:, 64:128], kx[:, 64:128],
                                     tmp[:, 64:128])
                nc.sync.dma_start(out=k_loc[32 * s:32 * s + 32, :], in_=kx)

    nc.finalize()
    return nc


# --------------------------------------------------------------------------
# launch 2: q GEMM + RoPE + qk + score assembly + mask + top-k
# --------------------------------------------------------------------------
def _build_l2():
    nc = bacc.Bacc()
    qr_sh = nc.declare_dram_parameter("qr_sh", [256, QR_DIM], F32, isOutput=False)
    wq = nc.declare_dram_parameter("wq", [H, 128, 12, 128], F32, isOutput=False)
    cs_own = nc.declare_dram_parameter("cs_own", [256, D], F32, isOutput=False)
    k_full = nc.declare_dram_parameter("k_full", [C, D], F32, isOutput=False)
    wts_own = nc.declare_dram_parameter("wts_own", [256, H], F32, isOutput=False)
    posm3 = nc.declare_dram_parameter("posm3", [256], F32, isOutput=False)
    out_idx = nc.declare_dram_parameter("out_idx", [256, TOPK], I32, isOutput=True)

    WIDTHS = (256, 512)  # candidate widths for (low tile j<=7, high tile)

    with tile.TileContext(nc) as tc, ExitStack() as ctx:
        const = ctx.enter_context(tc.tile_pool(name="const", bufs=1))
        work = ctx.enter_context(tc.tile_pool(name="work", bufs=2))
        tk = ctx.enter_context(tc.tile_pool(name="tk", bufs=2))

        ident = const.tile([128, 128], F32)
        make_identity(nc, ident)

        def tp(ps_out, in_sb):
            p = in_sb.shape[0]
            nc.tensor.transpose(ps_out, in_sb, ident[:p, :p])

        qrT = const.tile([128, 12, 256], F32)
        csT = const.tile([128, 256], F32)
        kT = const.tile([128, C], F32)
        with tc.tile_pool(name="stg", bufs=2) as stg, \
             tc.tile_pool(name="tps", bufs=2, space="PSUM") as tps:
            for tt in range(2):
                stage = stg.tile([128, QR_DIM], F32, tag="qstage")
                nc.sync.dma_start(out=stage,
                                  in_=qr_sh[tt * 128:(tt + 1) * 128, :])
                for kg in range(3):
                    ps = tps.tile([128, 512], F32, tag="tp")
                    for u in range(4):
                        kc = kg * 4 + u
                        tp(ps[:, u * 128:(u + 1) * 128],
                           stage[:, kc * 128:(kc + 1) * 128])
                    nc.scalar.copy(
                        qrT[:, kg * 4:kg * 4 + 4, tt * 128:(tt + 1) * 128],
                        ps.rearrange("p (u x) -> p u x", x=128))
            for tt in range(2):
                stage = stg.tile([128, D], F32, tag="cstage")
                nc.sync.dma_start(out=stage,
                                  in_=cs_own[tt * 128:(tt + 1) * 128, :])
                ps = tps.tile([128, 512], F32, tag="tp")
                tp(ps[:, :128], stage)
                nc.scalar.copy(csT[:, tt * 128:(tt + 1) * 128], ps[:, :128])
            kstage = const.tile([128, 4, D], F32)
            nc.sync.dma_start(out=kstage,
                              in_=k_full[:].rearrange("(a p) d -> p a d", p=128))
            for a in range(4):
                ps = tps.tile([128, 512], F32, tag="tp")
                tp(ps[:, :128], kstage[:, a, :])
                nc.scalar.copy(kT[:, a * 128:(a + 1) * 128], ps[:, :128])

        # cc = [cos;cos], ss = [-sin;sin] (partition moves via DMA only)
        cc = const.tile([128, 256], F32)
        ss = const.tile([128, 256], F32)
        nc.sync.dma_start(out=cc[0:64, :], in_=csT[0:64, :])
        nc.sync.dma_start(out=cc[64:128, :], in_=csT[0:64, :])
        nc.sync.dma_start(out=ss[0:64, :], in_=csT[64:128, :])
        nc.sync.dma_start(out=ss[64:128, :], in_=csT[64:128, :])
        nc.vector.tensor_scalar(ss[0:64, :], ss[0:64, :], -1.0, None,
                                op0=mybir.AluOpType.mult)

        wts_sb, pos_sb = [], []
        for tt in range(2):
            w = const.tile([128, H], F32, tag=f"wts{tt}")
            nc.sync.dma_start(out=w, in_=wts_own[tt * 128:(tt + 1) * 128, :])
            wts_sb.append(w)
            p = const.tile([128, 1], F32, tag=f"pos{tt}")
            nc.sync.dma_start(out=p, in_=posm3[tt * 128:(tt + 1) * 128])
            pos_sb.append(p)

        c4p = const.tile([128, C], F32)
        nc.gpsimd.iota(c4p, pattern=[[4, C]], base=0, channel_multiplier=0,
                       allow_small_or_imprecise_dtypes=True)
        c4f = const.tile([128, C], F32)
        nc.vector.tensor_scalar(c4f, c4p, -1.0, None, op0=mybir.AluOpType.mult)
        negs = const.tile([128, C], F32)
        nc.vector.memset(negs, NEG)
        neg1 = const.tile([128, TOPK], I32)
        nc.vector.memset(neg1, -1)

        acc = [const.tile([128, C], F32, tag=f"acc{tt}", name=f"acc{tt}") for tt in range(2)]

        def emit_topk(tt):
            W = WIDTHS[tt]
            cmp = work.tile([128, C], F32, tag="cmp", name="cmp")
            nc.vector.tensor_scalar(cmp[:, :W], c4f[:, :W], pos_sb[tt], None,
                                    op0=mybir.AluOpType.add)
            mbit = work.tile([128, C], U32, tag="mbit", name="mbit")
            nc.vector.tensor_scalar(mbit[:, :W], cmp[:, :W], 0.0, None,
                                    op0=mybir.AluOpType.is_lt)
            nc.vector.copy_predicated(acc[tt][:, :W], mbit[:, :W],
                                      negs[:, :W])

            idx = tk.tile([128, TOPK], U32, tag="idx", name="idx")
            vals = acc[tt]
            for it in range(32):
                mx = tk.tile([128, 8], F32, tag="mx", name="mx")
                nc.vector.max(out=mx, in_=vals[:, :W])
                nc.vector.max_index(out=idx[:, it * 8:(it + 1) * 8],
                                    in_max=mx, in_values=vals[:, :W])
                nc.vector.match_replace(out=vals[:, :W], in_to_replace=mx,
                                        in_values=vals[:, :W], imm_value=NEG)

            idx32 = tk.tile([128, TOPK], I32, tag="idx32", name="idx32")
            nc.vector.tensor_copy(idx32, idx)
            rmp = work.tile([128, TOPK], F32, tag="rmp", name="rmp")
            nc.vector.tensor_scalar(rmp, c4f[:, :TOPK], pos_sb[tt], None,
                                    op0=mybir.AluOpType.add)
            rbit = work.tile([128, TOPK], U32, tag="rbit", name="rbit")
            nc.vector.tensor_scalar(rbit, rmp, 0.0, None,
                                    op0=mybir.AluOpType.is_lt)
            nc.vector.copy_predicated(idx32, rbit, neg1)
            nc.sync.dma_start(out=out_idx[tt * 128:(tt + 1) * 128, :],
                              in_=idx32)

        qro_keep = ctx.enter_context(tc.tile_pool(name="qro_keep", bufs=H))
        qros = []
        with tc.tile_pool(name="wqp", bufs=3) as wqp, \
             tc.tile_pool(name="qro_p", bufs=2) as qro_p, \
             tc.tile_pool(name="qps", bufs=2, space="PSUM") as qps, \
             tc.tile_pool(name="qkps", bufs=2, space="PSUM") as qkps:
            for m in range(H):
                wqt = wqp.tile([128, 12, 128], F32, tag="wq")
                nc.sync.dma_start(out=wqt, in_=wq[m])
                ps_q = qps.tile([128, 256], F32, tag="qps")
                for kc in range(12):
                    nc.tensor.matmul(ps_q, wqt[:, kc, :], qrT[:, kc, :],
                                     start=(kc == 0), stop=(kc == 11))
                q_sb = qro_p.tile([128, 256], F32, tag="q_sb")
                nc.scalar.copy(q_sb, ps_q)
                q_sw = qro_p.tile([128, 256], F32, tag="q_sw")
                nc.sync.dma_start(out=q_sw[0:64, :], in_=q_sb[64:128, :])
                nc.sync.dma_start(out=q_sw[64:128, :], in_=q_sb[0:64, :])
                qro = qro_keep.tile([128, 256], F32, tag="qro")
                tmp = qro_p.tile([128, 256], F32, tag="qtmp")
                nc.vector.tensor_mul(qro, q_sb, cc)
                nc.vector.tensor_mul(tmp, q_sw, ss)
                nc.vector.tensor_add(qro, qro, tmp)
                qros.append(qro)

            def emit_qk(tt, m, qro):
                Wtt = WIDTHS[tt]
                ps_qk = qkps.tile([128, Wtt], F32, tag=f"qkps{tt}",
                                  name=f"ps_qk{tt}")
                nc.tensor.matmul(ps_qk, qro[:, tt * 128:(tt + 1) * 128],
                                 kT[:, :Wtt], start=True, stop=True)
                if m == 0:
                    nc.vector.tensor_scalar(
                        acc[tt][:, :Wtt], ps_qk, 0.0, wts_sb[tt][:, m:m + 1],
                        op0=mybir.AluOpType.max, op1=mybir.AluOpType.mult)
                else:
                    rl = work.tile([128, Wtt], F32, tag=f"rl{tt}",
                                   name=f"rl{tt}")
                    nc.vector.tensor_scalar(
                        rl, ps_qk, 0.0, wts_sb[tt][:, m:m + 1],
                        op0=mybir.AluOpType.max, op1=mybir.AluOpType.mult)
                    nc.vector.tensor_add(acc[tt][:, :Wtt],
                                         acc[tt][:, :Wtt], rl)

            for m in range(H):
                emit_qk(0, m, qros[m])
            emit_topk(0)
            for m in range(H):
                emit_qk(1, m, qros[m])
            emit_topk(1)

    nc.finalize()
    return nc


def _get(name):
    if name not in _cache:
        _cache[name] = _build_l1() if name == "l1" else _build_l2()
    return _cache[name]


def kernel(hidden_states, qr, positions, W_fused, Wq, Wproj, ape, rms_weight,
           cos_sin_cache, _timing=None):
    hidden_states = np.asarray(hidden_states, np.float32)
    qr = np.asarray(qr, np.float32)
    positions = np.asarray(positions, np.int32)
    W_fused = np.asarray(W_fused, np.float32)
    Wq = np.asarray(Wq, np.float32)
    Wproj = np.asarray(Wproj, np.float32)
    ape = np.asarray(ape, np.float32)
    rms_weight = np.asarray(rms_weight, np.float32)
    cos_sin_cache = np.asarray(cos_sin_cache, np.float32)

    wcomb = np.ascontiguousarray(
        np.concatenate([W_fused.T, Wproj.T], axis=1))          # [7168, 576]
    wq_pre = np.ascontiguousarray(
        Wq.reshape(H, 128, 12, 128).transpose(0, 3, 2, 1))     # [m, kk, kc, mm]

    cores = list(range(NC))
    trace = _timing is not None

    in1 = []
    for i in cores:
        rows = []
        for j in PAIRS[i]:
            lo = 128 * j - 4
            if lo < 0:
                blk = np.zeros((132, HID), np.float32)
                blk[4:] = hidden_states[0:128 * j + 128]
            else:
                blk = hidden_states[lo:128 * j + 128]
            rows.append(blk)
        hidden_halo = np.ascontiguousarray(np.concatenate(rows, axis=0))
        cs_rows = np.concatenate(
            [cos_sin_cache[32 * j:32 * j + 32] for j in PAIRS[i]], axis=0)
        hfl = np.array([NEG if j == 0 else 0.0 for j in PAIRS[i]], np.float32)
        in1.append({
            "hidden": hidden_halo, "wcomb": wcomb, "ape": ape,
            "rmsw": rms_weight, "cs_k": np.ascontiguousarray(cs_rows),
            "haloflag": hfl,
        })
    r1 = run_bass_kernel_spmd(_get("l1"), in1, cores, trace=trace,
                              trace_cores=cores if trace else None)

    k_full = np.zeros((C, D), np.float32)
    wts = {}
    for i in cores:
        kl = r1.results[i]["k_loc"]
        for s, j in enumerate(PAIRS[i]):
            k_full[32 * j:32 * j + 32] = kl[32 * s:32 * s + 32]
        wts[i] = r1.results[i]["wts_own"]

    in2 = []
    for i in cores:
        sel = np.concatenate(
            [np.arange(128 * j, 128 * j + 128) for j in PAIRS[i]])
        in2.append({
            "qr_sh": np.ascontiguousarray(qr[sel]),
            "wq": wq_pre,
            "cs_own": np.ascontiguousarray(cos_sin_cache[positions[sel]]),
            "k_full": k_full,
            "wts_own": wts[i],
            "posm3": (positions[sel] - 3).astype(np.float32),
        })
    r2 = run_bass_kernel_spmd(_get("l2"), in2, cores, trace=trace,
                              trace_cores=cores if trace else None)

    out = np.empty((T, TOPK), np.int32)
    for i in cores:
        oi = r2.results[i]["out_idx"]
        for s, j in enumerate(PAIRS[i]):
            out[128 * j:128 * j + 128] = oi[128 * s:128 * s + 128]

    if _timing is not None:
        _timing["l1"] = r1
        _timing["l2"] = r2
    return out



# revision 2
# speedup vs baseline: 1.0466x; 1.0466x over previous
"""DeepseekV4 indexer kernel for 8 trn2 NeuronCores (Bass/Tile) — v2.

vs baseline:
  - q GEMM runs in float32r (11-bit mantissa, 1 cyc/row at N>=512) with the
    token-strip as the stationary operand: 384 N=512 matmuls + 144 LDWs
    instead of 768 N=256 fp32 matmuls (4 cyc/row) + 768 LDWs.
    End-to-end index mismatch from this is 1.36e-2 (deterministic), within
    the 2e-2 gate.  All other GEMMs stay fp32.
  - q comes out token-major [t, h*128+d]; RoPE is done in that layout with
    strided column views (no partition-swap DMAs), then per-head PE
    transposes produce qroT [d, t] for the fp32 qk matmuls.
  - qk / relu-accum / top-k use exact causal widths: token tile j only ever
    scores against compressed keys c < 32*(j+1), and needs only
    min(4*(j+1), 32) top-8 extraction iterations.
  - relu moved to the Scalar engine; the weighted accumulation over heads is
    a single DVE scalar_tensor_tensor per head.
"""
import sys
sys.path.insert(0, '/opt/trn_rl_repo')

from contextlib import ExitStack

import numpy as np

import concourse.bass as bass
import concourse.bacc as bacc
import concourse.tile as tile
from concourse import mybir
from concourse.bass_utils import run_bass_kernel_spmd
from concourse.masks import make_identity

T, HID, QR_DIM, H, D, TOPK, R = 2048, 7168, 1536, 64, 128, 256, 4
C = T // R
NC = 8
EPS = 1e-6
F32 = mybir.dt.float32
F32R = mybir.dt.float32r
I32 = mybir.dt.int32
U32 = mybir.dt.uint32
WTS_SCALE = float(H ** -0.5) * float(D ** -0.5)  # folds q's D**-0.5 into wts
NEG = -1e30

PAIRS = [(i, 15 - i) for i in range(NC)]  # token tiles owned by core i

_cache = {}


# --------------------------------------------------------------------------
# launch 1: compressor -> per-core compressed K (64 rows) + head weights
# (unchanged from baseline except cosmetics)
# --------------------------------------------------------------------------
def _build_l1():
    nc = bacc.Bacc()
    hidden = nc.declare_dram_parameter("hidden", [264, HID], F32, isOutput=False)
    wcomb = nc.declare_dram_parameter("wcomb", [HID, 576], F32, isOutput=False)
    ape = nc.declare_dram_parameter("ape", [8, D], F32, isOutput=False)
    rmsw = nc.declare_dram_parameter("rmsw", [D], F32, isOutput=False)
    cs_k = nc.declare_dram_parameter("cs_k", [64, D], F32, isOutput=False)
    haloflag = nc.declare_dram_parameter("haloflag", [2], F32, isOutput=False)
    k_loc = nc.declare_dram_parameter("k_loc", [64, D], F32, isOutput=True)
    wts_own = nc.declare_dram_parameter("wts_own", [256, H], F32, isOutput=True)

    with tile.TileContext(nc) as tc, ExitStack() as ctx:
        const = ctx.enter_context(tc.tile_pool(name="const", bufs=1))
        big = ctx.enter_context(tc.tile_pool(name="big", bufs=1))
        work = ctx.enter_context(tc.tile_pool(name="work", bufs=2))

        ident = const.tile([128, 128], F32)
        make_identity(nc, ident)

        def tp(ps_out, in_sb):
            p = in_sb.shape[0]
            nc.tensor.transpose(ps_out, in_sb, ident[:p, :p])

        # ---- hiddenT [128, 56, 264] via PE transposes ----
        hidT = big.tile([128, 56, 264], F32)
        with tc.tile_pool(name="stg", bufs=2) as stg, \
             tc.tile_pool(name="tpsA", bufs=2, space="PSUM") as tpsA:
            for (t0, rows) in [(0, 128), (128, 128), (256, 8)]:
                stage = stg.tile([128, HID], F32, tag="stage")
                nc.sync.dma_start(out=stage[:rows, :], in_=hidden[t0:t0 + rows, :])
                for kg in range(14):
                    ps = tpsA.tile([128, 512], F32, tag="tp")
                    for u in range(4):
                        kc = kg * 4 + u
                        tp(ps[:, u * 128:u * 128 + rows],
                           stage[:rows, kc * 128:(kc + 1) * 128])
                    sv = ps.rearrange("p (u x) -> p u x", x=128)[:, :, :rows]
                    nc.scalar.copy(hidT[:, kg * 4:kg * 4 + 4, t0:t0 + rows], sv)

        # ---- fused GEMM: kv_scoreT [4x128, 264] + wtsT [64, 264] ----
        kvt = []
        wts_sb = work.tile([64, 264], F32, tag="wts_sb")
        with tc.tile_pool(name="wstg", bufs=3) as wstg, \
             tc.tile_pool(name="gps", bufs=1, space="PSUM") as gps:
            kvps = [gps.tile([128, 264], F32, tag=f"kvps{m}", name=f"kvps{m}") for m in range(4)]
            wtsps = gps.tile([64, 264], F32, tag="wtsps")
            for kc in range(56):
                wt = wstg.tile([128, 576], F32, tag="wcomb")
                nc.sync.dma_start(out=wt, in_=wcomb[kc * 128:(kc + 1) * 128, :])
                for m in range(4):
                    nc.tensor.matmul(kvps[m], wt[:, m * 128:(m + 1) * 128],
                                     hidT[:, kc, :], start=(kc == 0),
                                     stop=(kc == 55))
                nc.tensor.matmul(wtsps, wt[:, 512:576], hidT[:, kc, :],
                                 start=(kc == 0), stop=(kc == 55))
            for m in range(4):
                t = work.tile([128, 264], F32, tag=f"kvt{m}")
                nc.scalar.copy(t, kvps[m])
                kvt.append(t)
            nc.scalar.mul(wts_sb, wtsps, WTS_SCALE)
        kv_old, kv_new, sc_old, sc_new = kvt

        with tc.tile_pool(name="tpsB", bufs=2, space="PSUM") as tpsB:
            # wts -> [t, h] and out
            for s in range(2):
                ps = tpsB.tile([128, 64], F32, tag="wtp")
                tp(ps, wts_sb[:, 4 + 132 * s:132 + 132 * s])
                ob = work.tile([128, 64], F32, tag="wob")
                nc.scalar.copy(ob, ps)
                nc.sync.dma_start(out=wts_own[128 * s:128 * (s + 1), :], in_=ob)

            # ape transposed + replicated [128, 32, 8]
            ape_st = work.tile([8, D], F32, tag="ape_st")
            nc.sync.dma_start(out=ape_st, in_=ape[:])
            aps = tpsB.tile([128, 8], F32, tag="apetp")
            tp(aps, ape_st)
            apeT = const.tile([128, 8], F32)
            nc.scalar.copy(apeT, aps)
            ape_rep = const.tile([128, 32, 8], F32)
            for g in range(32):
                nc.vector.tensor_copy(ape_rep[:, g, :], apeT)

            # rms weight replicated [32, 128]
            rms_rep = const.tile([32, D], F32)
            nc.sync.dma_start(out=rms_rep, in_=bass.AP(
                tensor=rmsw, offset=0, ap=[[0, 32], [1, D]]))

            cs_st = []
            for s in range(2):
                cst = const.tile([32, D], F32, tag=f"cs{s}", name=f"cs{s}")
                nc.sync.dma_start(out=cst, in_=cs_k[32 * s:32 * s + 32, :])
                cs_st.append(cst)

            hf = []
            for s in range(2):
                h = const.tile([128, 1], F32, tag=f"hf{s}")
                nc.sync.dma_start(out=h, in_=bass.AP(
                    tensor=haloflag, offset=s, ap=[[0, 128], [1, 1]]))
                hf.append(h)

            for s in range(2):
                o = 132 * s
                gates = work.tile([128, 32, 8], F32, tag="gates")
                so_v = sc_old[:, o:o + 128].rearrange("p (g x) -> p g x", x=4)
                sn_v = sc_new[:, o + 4:o + 132].rearrange("p (g x) -> p g x", x=4)
                ko_v = kv_old[:, o:o + 128].rearrange("p (g x) -> p g x", x=4)
                kn_v = kv_new[:, o + 4:o + 132].rearrange("p (g x) -> p g x", x=4)
                nc.vector.tensor_add(gates[:, :, 0:4], so_v, ape_rep[:, :, 0:4])
                nc.vector.tensor_add(gates[:, :, 4:8], sn_v, ape_rep[:, :, 4:8])
                # first group's old slots += -1e30 when strip starts at t=0
                nc.vector.tensor_scalar(gates[:, 0, 0:4], gates[:, 0, 0:4],
                                        hf[s], None, op0=mybir.AluOpType.add)
                gmax = work.tile([128, 32], F32, tag="gmax")
                nc.vector.reduce_max(gmax, gates, axis=mybir.AxisListType.X)
                nc.vector.tensor_sub(gates, gates,
                                     gmax.to_broadcast([128, 32, 8]))
                ex = work.tile([128, 32, 8], F32, tag="ex")
                nc.scalar.activation(ex, gates, mybir.ActivationFunctionType.Exp)
                den = work.tile([128, 32], F32, tag="den")
                nc.vector.reduce_sum(den, ex, axis=mybir.AxisListType.X)
                rec = work.tile([128, 32], F32, tag="rec")
                nc.vector.reciprocal(rec, den)
                w8 = work.tile([128, 32, 8], F32, tag="w8")
                nc.vector.tensor_mul(w8, ex, rec.to_broadcast([128, 32, 8]))
                prod = work.tile([128, 32, 8], F32, tag="prod")
                nc.vector.tensor_mul(prod[:, :, 0:4], w8[:, :, 0:4], ko_v)
                nc.vector.tensor_mul(prod[:, :, 4:8], w8[:, :, 4:8], kn_v)
                comp = work.tile([128, 32], F32, tag="comp")
                nc.vector.reduce_sum(comp, prod, axis=mybir.AxisListType.X)

                cps = tpsB.tile([32, 128], F32, tag="ctp")
                tp(cps, comp)
                compT = work.tile([32, D], F32, tag="compT")
                nc.scalar.copy(compT, cps)

                # RMSNorm over d
                sq = work.tile([32, D], F32, tag="sq")
                nc.vector.tensor_mul(sq, compT, compT)
                ssum = work.tile([32, 1], F32, tag="ssum")
                nc.vector.reduce_sum(ssum, sq, axis=mybir.AxisListType.X)
                nc.vector.tensor_scalar(ssum, ssum, 1.0 / D, EPS,
                                        op0=mybir.AluOpType.mult,
                                        op1=mybir.AluOpType.add)
                rt = work.tile([32, 1], F32, tag="rt")
                nc.scalar.sqrt(rt, ssum)
                rs = work.tile([32, 1], F32, tag="rs")
                nc.vector.reciprocal(rs, rt)
                nc.vector.tensor_scalar(compT, compT, rs, None,
                                        op0=mybir.AluOpType.mult)
                nc.vector.tensor_mul(compT, compT, rms_rep)

                # RoPE at compressed positions (all tiles at base partition 0)
                co = cs_st[s][:, 0:64]
                si = cs_st[s][:, 64:128]
                x1 = compT[:, 0:64]
                x2 = compT[:, 64:128]
                tmp = work.tile([32, D], F32, tag="ktmp")
                kx = work.tile([32, D], F32, tag="kx")
                nc.vector.tensor_mul(kx[:, 0:64], x1, co)
                nc.vector.tensor_mul(tmp[:, 0:64], x2, si)
                nc.vector.tensor_sub(kx[:, 0:64], kx[:, 0:64], tmp[:, 0:64])
                nc.vector.tensor_mul(kx[:, 64:128], x2, co)
                nc.vector.tensor_mul(tmp[:, 64:128], x1, si)
                nc.vector.tensor_add(kx[:, 64:128], kx[:, 64:128],
                                     tmp[:, 64:128])
                nc.sync.dma_start(out=k_loc[32 * s:32 * s + 32, :], in_=kx)

    nc.finalize()
    return nc


# --------------------------------------------------------------------------
# launch 2: q GEMM (f32r, token-stationary) + RoPE + qk + scores + top-k
# --------------------------------------------------------------------------
def _build_l2(widths):
    """widths: (W_A, W_B) causal widths of this core's two token tiles."""
    nc = bacc.Bacc()
    qr_sh = nc.declare_dram_parameter("qr_sh", [256, QR_DIM], F32, isOutput=False)
    # Wq.T reshaped [12 kc, 128 qr, 8192 m] and rounded to f32r on device read
    wqT = nc.declare_dram_parameter("wqT", [12, 128, H * D], F32R, isOutput=False)
    cs_own = nc.declare_dram_parameter("cs_own", [256, D], F32, isOutput=False)
    k_full = nc.declare_dram_parameter("k_full", [C, D], F32, isOutput=False)
    wts_own = nc.declare_dram_parameter("wts_own", [256, H], F32, isOutput=False)
    posm3 = nc.declare_dram_parameter("posm3", [256], F32, isOutput=False)
    out_idx = nc.declare_dram_parameter("out_idx", [256, TOPK], I32, isOutput=True)

    ITERS = tuple(min((w // 32) * 4, 32) for w in widths)

    with tile.TileContext(nc) as tc, ExitStack() as ctx:
        const = ctx.enter_context(tc.tile_pool(name="const", bufs=1))
        work = ctx.enter_context(tc.tile_pool(name="work", bufs=2))
        tk = ctx.enter_context(tc.tile_pool(name="tk", bufs=2))

        ident = const.tile([128, 128], F32)
        make_identity(nc, ident)

        def tp(ps_out, in_sb):
            p = in_sb.shape[0]
            nc.tensor.transpose(ps_out, in_sb, ident[:p, :p])

        # ---- prep: qrT (f32r), kT, cos/sin strips, wts, pos ----
        qrT = const.tile([128, 12, 256], F32R)
        kT = const.tile([128, C], F32)
        with tc.tile_pool(name="stg", bufs=2) as stg, \
             tc.tile_pool(name="tps", bufs=2, space="PSUM") as tps:
            for tt in range(2):
                stage = stg.tile([128, QR_DIM], F32, tag="qstage")
                nc.sync.dma_start(out=stage,
                                  in_=qr_sh[tt * 128:(tt + 1) * 128, :])
                for kg in range(3):
                    ps = tps.tile([128, 512], F32, tag="tp")
                    for u in range(4):
                        kc = kg * 4 + u
                        tp(ps[:, u * 128:(u + 1) * 128],
                           stage[:, kc * 128:(kc + 1) * 128])
                    nc.scalar.copy(
                        qrT[:, kg * 4:kg * 4 + 4, tt * 128:(tt + 1) * 128],
                        ps.rearrange("p (u x) -> p u x", x=128))
            kstage = const.tile([128, 4, D], F32)
            nc.sync.dma_start(out=kstage,
                              in_=k_full[:].rearrange("(a p) d -> p a d", p=128))
            for a in range(4):
                ps = tps.tile([128, 512], F32, tag="tp")
                tp(ps[:, :128], kstage[:, a, :])
                nc.scalar.copy(kT[:, a * 128:(a + 1) * 128], ps[:, :128])

        cs_sb, wts_sb, pos_sb = [], [], []
        for tt in range(2):
            csb = const.tile([128, D], F32, tag=f"cs{tt}", name=f"cs{tt}")
            nc.sync.dma_start(out=csb, in_=cs_own[tt * 128:(tt + 1) * 128, :])
            cs_sb.append(csb)
            w = const.tile([128, H], F32, tag=f"wts{tt}", name=f"wts{tt}")
            nc.sync.dma_start(out=w, in_=wts_own[tt * 128:(tt + 1) * 128, :])
            wts_sb.append(w)
            p = const.tile([128, 1], F32, tag=f"pos{tt}", name=f"pos{tt}")
            nc.sync.dma_start(out=p, in_=posm3[tt * 128:(tt + 1) * 128])
            pos_sb.append(p)

        c4p = const.tile([128, C], F32)
        nc.gpsimd.iota(c4p, pattern=[[4, C]], base=0, channel_multiplier=0,
                       allow_small_or_imprecise_dtypes=True)
        c4f = const.tile([128, C], F32)
        nc.vector.tensor_scalar(c4f, c4p, -1.0, None,
                                op0=mybir.AluOpType.mult)
        negs = const.tile([128, C], F32)
        nc.vector.memset(negs, NEG)
        neg1 = const.tile([128, TOPK], I32)
        nc.vector.memset(neg1, -1)

        # ---- q GEMM (f32r) + RoPE + per-head transposes ----
        # q[t, m] accumulated in PSUM over 12 qr chunks; 3 m-chunks of 512
        # in flight (6 PSUM banks for both strips) + 1 transpose bank.
        qroT = const.tile([128, H, 256], F32)   # [d, h, t]
        acc = [const.tile([128, widths[tt]], F32, tag=f"acc{tt}",
                          name=f"acc{tt}") for tt in range(2)]

        with tc.tile_pool(name="wqp", bufs=8) as wqp, \
             tc.tile_pool(name="qwork", bufs=3) as qwork, \
             tc.tile_pool(name="qps", bufs=1, space="PSUM") as qps, \
             tc.tile_pool(name="tps2", bufs=2, space="PSUM") as tps2:
            MCG = [(0, 3), (3, 3), (6, 3), (9, 3), (12, 3), (15, 1)]
            for (mc0, nmc) in MCG:
                psq = [[qps.tile([128, 512], F32, tag=f"q{tt}{i}",
                                 name=f"psq{tt}{i}")
                        for i in range(nmc)] for tt in range(2)]
                for kc in range(12):
                    wtiles = []
                    for i in range(nmc):
                        mc = mc0 + i
                        wt = wqp.tile([128, 512], F32R, tag="wq")
                        nc.sync.dma_start(
                            out=wt, in_=wqT[kc][:, mc * 512:(mc + 1) * 512])
                        wtiles.append(wt)
                    for tt in range(2):
                        lhs = qrT[:, kc, tt * 128:(tt + 1) * 128]
                        for i in range(nmc):
                            nc.tensor.matmul(psq[tt][i], lhs, wtiles[i],
                                             start=(kc == 0), stop=(kc == 11))
                for tt in range(2):
                    for i in range(nmc):
                        mc = mc0 + i
                        q_sb = qwork.tile([128, 4, 128], F32, tag="q_sb")
                        nc.scalar.copy(q_sb, psq[tt][i].rearrange(
                            "p (h x) -> p h x", x=128))
                        # RoPE in [t, h, d] layout (NeoX halves are columns)
                        co = cs_sb[tt][:, 0:64].unsqueeze(1).to_broadcast(
                            [128, 4, 64])
                        si = cs_sb[tt][:, 64:128].unsqueeze(1).to_broadcast(
                            [128, 4, 64])
                        x1 = q_sb[:, :, 0:64]
                        x2 = q_sb[:, :, 64:128]
                        qro = qwork.tile([128, 4, 128], F32, tag="qro")
                        tmp = qwork.tile([128, 4, 128], F32, tag="qtmp")
                        nc.vector.tensor_mul(qro[:, :, 0:64], x1, co)
                        nc.vector.tensor_mul(tmp[:, :, 0:64], x2, si)
                        nc.vector.tensor_sub(qro[:, :, 0:64],
                                             qro[:, :, 0:64], tmp[:, :, 0:64])
                        nc.vector.tensor_mul(qro[:, :, 64:128], x2, co)
                        nc.vector.tensor_mul(tmp[:, :, 64:128], x1, si)
                        nc.vector.tensor_add(qro[:, :, 64:128],
                                             qro[:, :, 64:128],
                                             tmp[:, :, 64:128])
                        # transpose 4 heads -> qroT[d, h, t-strip]
                        pst = tps2.tile([128, 4, 128], F32, tag="tp4")
                        for hh in range(4):
                            tp(pst[:, hh, :], qro[:, hh, :])
                        nc.scalar.copy(
                            qroT[:, 4 * mc:4 * mc + 4, tt * 128:(tt + 1) * 128],
                            pst)

        # ---- qk (fp32) + relu (scalar) + weighted accum (DVE) ----
        def emit_scores(tt):
            Wt = widths[tt]
            with tc.tile_pool(name=f"qkps{tt}", bufs=3, space="PSUM") as qkps, \
                 tc.tile_pool(name=f"rlp{tt}", bufs=3) as rlp:
                for h in range(H):
                    ps_qk = qkps.tile([128, Wt], F32, tag="qk", name=f"qk{tt}")
                    nc.tensor.matmul(ps_qk, qroT[:, h, tt * 128:(tt + 1) * 128],
                                     kT[:, :Wt], start=True, stop=True)
                    rl = rlp.tile([128, Wt], F32, tag="rl", name=f"rl{tt}")
                    nc.scalar.activation(rl, ps_qk,
                                         mybir.ActivationFunctionType.Relu)
                    wcol = wts_sb[tt][:, h:h + 1]
                    if h == 0:
                        nc.vector.tensor_scalar(acc[tt], rl, wcol, None,
                                                op0=mybir.AluOpType.mult)
                    else:
                        nc.vector.scalar_tensor_tensor(
                            out=acc[tt], in0=rl, scalar=wcol, in1=acc[tt],
                            op0=mybir.AluOpType.mult,
                            op1=mybir.AluOpType.add)

        def emit_topk(tt):
            Wt = widths[tt]
            iters = ITERS[tt]
            nk = 8 * iters
            # causal mask: c >= num_comp -> NEG
            cmp = work.tile([128, C], F32, tag="cmp", name="cmp")
            nc.vector.tensor_scalar(cmp[:, :Wt], c4f[:, :Wt], pos_sb[tt], None,
                                    op0=mybir.AluOpType.add)
            mbit = work.tile([128, C], U32, tag="mbit", name="mbit")
            nc.vector.tensor_scalar(mbit[:, :Wt], cmp[:, :Wt], 0.0, None,
                                    op0=mybir.AluOpType.is_lt)
            nc.vector.copy_predicated(acc[tt][:, :Wt], mbit[:, :Wt],
                                      negs[:, :Wt])

            idx = tk.tile([128, TOPK], U32, tag="idx", name="idx")
            vals = acc[tt]
            for it in range(iters):
                mx = tk.tile([128, 8], F32, tag="mx", name="mx")
                nc.vector.max(out=mx, in_=vals[:, :Wt])
                nc.vector.max_index(out=idx[:, it * 8:(it + 1) * 8],
                                    in_max=mx, in_values=vals[:, :Wt])
                nc.vector.match_replace(out=vals[:, :Wt], in_to_replace=mx,
                                        in_values=vals[:, :Wt], imm_value=NEG)

            idx32 = tk.tile([128, TOPK], I32, tag="idx32", name="idx32")
            if nk < TOPK:
                nc.vector.memset(idx32[:, nk:], -1)
            nc.vector.tensor_copy(idx32[:, :nk], idx[:, :nk])
            rmp = work.tile([128, TOPK], F32, tag="rmp", name="rmp")
            nc.vector.tensor_scalar(rmp[:, :nk], c4f[:, :nk], pos_sb[tt], None,
                                    op0=mybir.AluOpType.add)
            rbit = work.tile([128, TOPK], U32, tag="rbit", name="rbit")
            nc.vector.tensor_scalar(rbit[:, :nk], rmp[:, :nk], 0.0, None,
                                    op0=mybir.AluOpType.is_lt)
            nc.vector.copy_predicated(idx32[:, :nk], rbit[:, :nk],
                                      neg1[:, :nk])
            nc.sync.dma_start(out=out_idx[tt * 128:(tt + 1) * 128, :],
                              in_=idx32)

        emit_scores(0)
        emit_topk(0)
        emit_scores(1)
        emit_topk(1)

    nc.finalize()
    return nc


def _get(name, *args):
    key = (name, args)
    if key not in _cache:
        _cache[key] = _build_l1() if name == "l1" else _build_l2(*args)
    return _cache[key]


def kernel(hidden_states, qr, positions, W_fused, Wq, Wproj, ape, rms_weight,
           cos_sin_cache, _timing=None):
    hidden_states = np.asarray(hidden_states, np.float32)
    qr = np.asarray(qr, np.float32)
    positions = np.asarray(positions, np.int32)
    W_fused = np.asarray(W_fused, np.float32)
    Wq = np.asarray(Wq, np.float32)
    Wproj = np.asarray(Wproj, np.float32)
    ape = np.asarray(ape, np.float32)
    rms_weight = np.asarray(rms_weight, np.float32)
    cos_sin_cache = np.asarray(cos_sin_cache, np.float32)

    wcomb = np.ascontiguousarray(
        np.concatenate([W_fused.T, Wproj.T], axis=1))          # [7168, 576]
    wqT = np.ascontiguousarray(
        Wq.T.reshape(12, 128, H * D))                          # [kc, qr, m]

    cores = list(range(NC))
    trace = _timing is not None

    in1 = []
    for i in cores:
        rows = []
        for j in PAIRS[i]:
            lo = 128 * j - 4
            if lo < 0:
                blk = np.zeros((132, HID), np.float32)
                blk[4:] = hidden_states[0:128 * j + 128]
            else:
                blk = hidden_states[lo:128 * j + 128]
            rows.append(blk)
        hidden_halo = np.ascontiguousarray(np.concatenate(rows, axis=0))
        cs_rows = np.concatenate(
            [cos_sin_cache[32 * j:32 * j + 32] for j in PAIRS[i]], axis=0)
        hfl = np.array([NEG if j == 0 else 0.0 for j in PAIRS[i]], np.float32)
        in1.append({
            "hidden": hidden_halo, "wcomb": wcomb, "ape": ape,
            "rmsw": rms_weight, "cs_k": np.ascontiguousarray(cs_rows),
            "haloflag": hfl,
        })
    r1 = run_bass_kernel_spmd(_get("l1"), in1, cores, trace=trace,
                              trace_cores=cores if trace else None)

    k_full = np.zeros((C, D), np.float32)
    wts = {}
    for i in cores:
        kl = r1.results[i]["k_loc"]
        for s, j in enumerate(PAIRS[i]):
            k_full[32 * j:32 * j + 32] = kl[32 * s:32 * s + 32]
        wts[i] = r1.results[i]["wts_own"]

    # one SPMD program for all cores: tile A of core i covers tokens
    # 128i..128i+127 (causal width <= 256), tile B covers the mirror tile
    # (width <= 512); masked-NEG columns make the uniform widths correct.
    in2 = []
    for i in cores:
        sel = np.concatenate(
            [np.arange(128 * j, 128 * j + 128) for j in PAIRS[i]])
        in2.append({
            "qr_sh": np.ascontiguousarray(qr[sel]),
            "wqT": wqT,
            "cs_own": np.ascontiguousarray(cos_sin_cache[positions[sel]]),
            "k_full": k_full,
            "wts_own": wts[i],
            "posm3": (positions[sel] - 3).astype(np.float32),
        })
    r2 = run_bass_kernel_spmd(_get("l2", (256, 512)), in2, cores, trace=trace,
                              trace_cores=cores if trace else None)

    out = np.empty((T, TOPK), np.int32)
    for i in cores:
        oi = r2.results[i]["out_idx"]
        for s, j in enumerate(PAIRS[i]):
            out[128 * j:128 * j + 128] = oi[128 * s:128 * s + 128]

    if _timing is not None:
        _timing["l1"] = r1
        _timing["l2"] = r2
    return out


# revision 4
# speedup vs baseline: 1.1328x; 1.0823x over previous
"""DeepseekV4 indexer kernel for 8 trn2 NeuronCores (Bass/Tile) — v2.

vs baseline:
  - q GEMM runs in float32r (11-bit mantissa, 1 cyc/row at N>=512) with the
    token-strip as the stationary operand: 384 N=512 matmuls + 144 LDWs
    instead of 768 N=256 fp32 matmuls (4 cyc/row) + 768 LDWs.
    End-to-end index mismatch from this is 1.36e-2 (deterministic), within
    the 2e-2 gate.  All other GEMMs stay fp32.
  - q comes out token-major [t, h*128+d]; RoPE is done in that layout with
    strided column views (no partition-swap DMAs), then per-head PE
    transposes produce qroT [d, t] for the fp32 qk matmuls.
  - qk / relu-accum / top-k use exact causal widths: token tile j only ever
    scores against compressed keys c < 32*(j+1), and needs only
    min(4*(j+1), 32) top-8 extraction iterations.
  - relu moved to the Scalar engine; the weighted accumulation over heads is
    a single DVE scalar_tensor_tensor per head.
"""
import sys
sys.path.insert(0, '/opt/trn_rl_repo')

from contextlib import ExitStack

import numpy as np

import concourse.bass as bass
import concourse.bacc as bacc
import concourse.tile as tile
from concourse import mybir
from concourse.bass_utils import run_bass_kernel_spmd
from concourse.masks import make_identity

T, HID, QR_DIM, H, D, TOPK, R = 2048, 7168, 1536, 64, 128, 256, 4
C = T // R
NC = 8
EPS = 1e-6
F32 = mybir.dt.float32
F32R = mybir.dt.float32r
I32 = mybir.dt.int32
U32 = mybir.dt.uint32
WTS_SCALE = float(H ** -0.5) * float(D ** -0.5)  # folds q's D**-0.5 into wts
NEG = -1e30

PAIRS = [(i, 15 - i) for i in range(NC)]  # token tiles owned by core i

_cache = {}


# --------------------------------------------------------------------------
# launch 1: compressor -> per-core compressed K (64 rows) + head weights
# (unchanged from baseline except cosmetics)
# --------------------------------------------------------------------------
def _build_l1():
    nc = bacc.Bacc()
    hidden = nc.declare_dram_parameter("hidden", [264, HID], F32, isOutput=False)
    wcomb = nc.declare_dram_parameter("wcomb", [HID, 576], F32, isOutput=False)
    ape = nc.declare_dram_parameter("ape", [8, D], F32, isOutput=False)
    rmsw = nc.declare_dram_parameter("rmsw", [D], F32, isOutput=False)
    cs_k = nc.declare_dram_parameter("cs_k", [64, D], F32, isOutput=False)
    haloflag = nc.declare_dram_parameter("haloflag", [2], F32, isOutput=False)
    k_loc = nc.declare_dram_parameter("k_loc", [64, D], F32, isOutput=True)
    wts_own = nc.declare_dram_parameter("wts_own", [256, H], F32, isOutput=True)

    with tile.TileContext(nc) as tc, ExitStack() as ctx:
        const = ctx.enter_context(tc.tile_pool(name="const", bufs=1))
        big = ctx.enter_context(tc.tile_pool(name="big", bufs=1))
        work = ctx.enter_context(tc.tile_pool(name="work", bufs=2))

        ident = const.tile([128, 128], F32)
        make_identity(nc, ident)

        def tp(ps_out, in_sb):
            p = in_sb.shape[0]
            nc.tensor.transpose(ps_out, in_sb, ident[:p, :p])

        # ---- interleaved: per 512-col group, DMA hidden chunk ->
        # transpose -> 4 GEMM k-chunks (PE starts after the first 0.8 MB) ----
        # ---- fused GEMM: kv_scoreT [4x128, 264] + wtsT [64, 264] ----
        kvt = []
        wts_sb = work.tile([64, 264], F32, tag="wts_sb")
        with tc.tile_pool(name="stg", bufs=3) as stg, \
             tc.tile_pool(name="hidp", bufs=2) as hidp, \
             tc.tile_pool(name="tpsA", bufs=2, space="PSUM") as tpsA, \
             tc.tile_pool(name="wstg", bufs=3) as wstg, \
             tc.tile_pool(name="gps", bufs=1, space="PSUM") as gps:
            kvps = [gps.tile([128, 264], F32, tag=f"kvps{m}", name=f"kvps{m}") for m in range(4)]
            wtsps = gps.tile([64, 264], F32, tag="wtsps")
            for kg in range(14):
                hidT = hidp.tile([128, 4, 264], F32, tag="hidT")
                for (si, (t0, rows)) in enumerate([(0, 128), (128, 128), (256, 8)]):
                    stage = stg.tile([128, 512], F32, tag=f"st{si}",
                                     name=f"st{si}")
                    nc.sync.dma_start(
                        out=stage[:rows, :],
                        in_=hidden[t0:t0 + rows, kg * 512:(kg + 1) * 512])
                    ps = tpsA.tile([128, 512], F32, tag="tp")
                    for u in range(4):
                        tp(ps[:, u * 128:u * 128 + rows],
                           stage[:rows, u * 128:(u + 1) * 128])
                    sv = ps.rearrange("p (u x) -> p u x", x=128)[:, :, :rows]
                    nc.scalar.copy(hidT[:, :, t0:t0 + rows], sv)
                for u in range(4):
                    kc = kg * 4 + u
                    wt = wstg.tile([128, 576], F32, tag="wcomb")
                    nc.sync.dma_start(out=wt, in_=wcomb[kc * 128:(kc + 1) * 128, :])
                    for m in range(4):
                        nc.tensor.matmul(kvps[m], wt[:, m * 128:(m + 1) * 128],
                                         hidT[:, u, :], start=(kc == 0),
                                         stop=(kc == 55))
                    nc.tensor.matmul(wtsps, wt[:, 512:576], hidT[:, u, :],
                                     start=(kc == 0), stop=(kc == 55))
            for m in range(4):
                t = work.tile([128, 264], F32, tag=f"kvt{m}")
                nc.scalar.copy(t, kvps[m])
                kvt.append(t)
            nc.scalar.mul(wts_sb, wtsps, WTS_SCALE)
        kv_old, kv_new, sc_old, sc_new = kvt

        with tc.tile_pool(name="tpsB", bufs=2, space="PSUM") as tpsB:
            # wts -> [t, h] and out
            for s in range(2):
                ps = tpsB.tile([128, 64], F32, tag="wtp")
                tp(ps, wts_sb[:, 4 + 132 * s:132 + 132 * s])
                ob = work.tile([128, 64], F32, tag="wob")
                nc.scalar.copy(ob, ps)
                nc.sync.dma_start(out=wts_own[128 * s:128 * (s + 1), :], in_=ob)

            # ape transposed + replicated [128, 32, 8]
            ape_st = work.tile([8, D], F32, tag="ape_st")
            nc.sync.dma_start(out=ape_st, in_=ape[:])
            aps = tpsB.tile([128, 8], F32, tag="apetp")
            tp(aps, ape_st)
            apeT = const.tile([128, 8], F32)
            nc.scalar.copy(apeT, aps)
            ape_rep = const.tile([128, 32, 8], F32)
            for g in range(32):
                nc.vector.tensor_copy(ape_rep[:, g, :], apeT)

            # rms weight replicated [32, 128]
            rms_rep = const.tile([32, D], F32)
            nc.sync.dma_start(out=rms_rep, in_=bass.AP(
                tensor=rmsw, offset=0, ap=[[0, 32], [1, D]]))

            cs_st = []
            for s in range(2):
                cst = const.tile([32, D], F32, tag=f"cs{s}", name=f"cs{s}")
                nc.sync.dma_start(out=cst, in_=cs_k[32 * s:32 * s + 32, :])
                cs_st.append(cst)

            hf = []
            for s in range(2):
                h = const.tile([128, 1], F32, tag=f"hf{s}")
                nc.sync.dma_start(out=h, in_=bass.AP(
                    tensor=haloflag, offset=s, ap=[[0, 128], [1, 1]]))
                hf.append(h)

            for s in range(2):
                o = 132 * s
                gates = work.tile([128, 32, 8], F32, tag="gates")
                so_v = sc_old[:, o:o + 128].rearrange("p (g x) -> p g x", x=4)
                sn_v = sc_new[:, o + 4:o + 132].rearrange("p (g x) -> p g x", x=4)
                ko_v = kv_old[:, o:o + 128].rearrange("p (g x) -> p g x", x=4)
                kn_v = kv_new[:, o + 4:o + 132].rearrange("p (g x) -> p g x", x=4)
                nc.vector.tensor_add(gates[:, :, 0:4], so_v, ape_rep[:, :, 0:4])
                nc.vector.tensor_add(gates[:, :, 4:8], sn_v, ape_rep[:, :, 4:8])
                # first group's old slots += -1e30 when strip starts at t=0
                nc.vector.tensor_scalar(gates[:, 0, 0:4], gates[:, 0, 0:4],
                                        hf[s], None, op0=mybir.AluOpType.add)
                gmax = work.tile([128, 32], F32, tag="gmax")
                nc.vector.reduce_max(gmax, gates, axis=mybir.AxisListType.X)
                nc.vector.tensor_sub(gates, gates,
                                     gmax.to_broadcast([128, 32, 8]))
                ex = work.tile([128, 32, 8], F32, tag="ex")
                nc.scalar.activation(ex, gates, mybir.ActivationFunctionType.Exp)
                den = work.tile([128, 32], F32, tag="den")
                nc.vector.reduce_sum(den, ex, axis=mybir.AxisListType.X)
                rec = work.tile([128, 32], F32, tag="rec")
                nc.vector.reciprocal(rec, den)
                w8 = work.tile([128, 32, 8], F32, tag="w8")
                nc.vector.tensor_mul(w8, ex, rec.to_broadcast([128, 32, 8]))
                prod = work.tile([128, 32, 8], F32, tag="prod")
                nc.vector.tensor_mul(prod[:, :, 0:4], w8[:, :, 0:4], ko_v)
                nc.vector.tensor_mul(prod[:, :, 4:8], w8[:, :, 4:8], kn_v)
                comp = work.tile([128, 32], F32, tag="comp")
                nc.vector.reduce_sum(comp, prod, axis=mybir.AxisListType.X)

                cps = tpsB.tile([32, 128], F32, tag="ctp")
                tp(cps, comp)
                compT = work.tile([32, D], F32, tag="compT")
                nc.scalar.copy(compT, cps)

                # RMSNorm over d
                sq = work.tile([32, D], F32, tag="sq")
                nc.vector.tensor_mul(sq, compT, compT)
                ssum = work.tile([32, 1], F32, tag="ssum")
                nc.vector.reduce_sum(ssum, sq, axis=mybir.AxisListType.X)
                nc.vector.tensor_scalar(ssum, ssum, 1.0 / D, EPS,
                                        op0=mybir.AluOpType.mult,
                                        op1=mybir.AluOpType.add)
                rt = work.tile([32, 1], F32, tag="rt")
                nc.scalar.sqrt(rt, ssum)
                rs = work.tile([32, 1], F32, tag="rs")
                nc.vector.reciprocal(rs, rt)
                nc.vector.tensor_scalar(compT, compT, rs, None,
                                        op0=mybir.AluOpType.mult)
                nc.vector.tensor_mul(compT, compT, rms_rep)

                # RoPE at compressed positions (all tiles at base partition 0)
                co = cs_st[s][:, 0:64]
                si = cs_st[s][:, 64:128]
                x1 = compT[:, 0:64]
                x2 = compT[:, 64:128]
                tmp = work.tile([32, D], F32, tag="ktmp")
                kx = work.tile([32, D], F32, tag="kx")
                nc.vector.tensor_mul(kx[:, 0:64], x1, co)
                nc.vector.tensor_mul(tmp[:, 0:64], x2, si)
                nc.vector.tensor_sub(kx[:, 0:64], kx[:, 0:64], tmp[:, 0:64])
                nc.vector.tensor_mul(kx[:, 64:128], x2, co)
                nc.vector.tensor_mul(tmp[:, 64:128], x1, si)
                nc.vector.tensor_add(kx[:, 64:128], kx[:, 64:128],
                                     tmp[:, 64:128])
                nc.sync.dma_start(out=k_loc[32 * s:32 * s + 32, :], in_=kx)

    nc.finalize()
    return nc


# --------------------------------------------------------------------------
# launch 2: q GEMM (f32r, token-stationary) + RoPE + qk + scores + top-k
# --------------------------------------------------------------------------
def _build_l2(widths):
    """widths: (W_A, W_B) causal widths of this core's two token tiles."""
    nc = bacc.Bacc()
    qr_sh = nc.declare_dram_parameter("qr_sh", [256, QR_DIM], F32, isOutput=False)
    # Wq.T reshaped [12 kc, 128 qr, 8192 m] and rounded to f32r on device read
    wqT = nc.declare_dram_parameter("wqT", [12, 128, H * D], F32R, isOutput=False)
    cs_own = nc.declare_dram_parameter("cs_own", [256, D], F32, isOutput=False)
    k_full = nc.declare_dram_parameter("k_full", [C, D], F32, isOutput=False)
    wts_own = nc.declare_dram_parameter("wts_own", [256, H], F32, isOutput=False)
    posm3 = nc.declare_dram_parameter("posm3", [256], F32, isOutput=False)
    out_idx = nc.declare_dram_parameter("out_idx", [256, TOPK], I32, isOutput=True)

    ITERS = tuple(min((w // 32) * 4, 32) for w in widths)

    with tile.TileContext(nc) as tc, ExitStack() as ctx:
        const = ctx.enter_context(tc.tile_pool(name="const", bufs=1))
        work = ctx.enter_context(tc.tile_pool(name="work", bufs=2))
        tk = ctx.enter_context(tc.tile_pool(name="tk", bufs=2))

        ident = const.tile([128, 128], F32)
        make_identity(nc, ident)

        def tp(ps_out, in_sb):
            p = in_sb.shape[0]
            nc.tensor.transpose(ps_out, in_sb, ident[:p, :p])

        # ---- prep: qrT (f32r), kT, cos/sin strips, wts, pos ----
        qrT = const.tile([128, 12, 256], F32R)
        kT = const.tile([128, C], F32)
        with tc.tile_pool(name="stg", bufs=2) as stg, \
             tc.tile_pool(name="tps", bufs=2, space="PSUM") as tps:
            for tt in range(2):
                stage = stg.tile([128, QR_DIM], F32, tag="qstage")
                nc.sync.dma_start(out=stage,
                                  in_=qr_sh[tt * 128:(tt + 1) * 128, :])
                for kg in range(3):
                    ps = tps.tile([128, 512], F32, tag="tp")
                    for u in range(4):
                        kc = kg * 4 + u
                        tp(ps[:, u * 128:(u + 1) * 128],
                           stage[:, kc * 128:(kc + 1) * 128])
                    nc.scalar.copy(
                        qrT[:, kg * 4:kg * 4 + 4, tt * 128:(tt + 1) * 128],
                        ps.rearrange("p (u x) -> p u x", x=128))
            kstage = const.tile([128, 4, D], F32)
            nc.sync.dma_start(out=kstage,
                              in_=k_full[:].rearrange("(a p) d -> p a d", p=128))
            for a in range(4):
                ps = tps.tile([128, 512], F32, tag="tp")
                tp(ps[:, :128], kstage[:, a, :])
                nc.scalar.copy(kT[:, a * 128:(a + 1) * 128], ps[:, :128])

        cs_sb, wts_sb, pos_sb = [], [], []
        for tt in range(2):
            csb = const.tile([128, D], F32, tag=f"cs{tt}", name=f"cs{tt}")
            nc.sync.dma_start(out=csb, in_=cs_own[tt * 128:(tt + 1) * 128, :])
            cs_sb.append(csb)
            w = const.tile([128, H], F32, tag=f"wts{tt}", name=f"wts{tt}")
            nc.sync.dma_start(out=w, in_=wts_own[tt * 128:(tt + 1) * 128, :])
            wts_sb.append(w)
            p = const.tile([128, 1], F32, tag=f"pos{tt}", name=f"pos{tt}")
            nc.sync.dma_start(out=p, in_=posm3[tt * 128:(tt + 1) * 128])
            pos_sb.append(p)

        c4p = const.tile([128, C], F32)
        nc.gpsimd.iota(c4p, pattern=[[4, C]], base=0, channel_multiplier=0,
                       allow_small_or_imprecise_dtypes=True)
        c4f = const.tile([128, C], F32)
        nc.vector.tensor_scalar(c4f, c4p, -1.0, None,
                                op0=mybir.AluOpType.mult)
        negs = const.tile([128, C], F32)
        nc.vector.memset(negs, NEG)
        neg1 = const.tile([128, TOPK], I32)
        nc.vector.memset(neg1, -1)

        # ---- per-strip pipeline: q GEMM (f32r) -> RoPE -> transpose -> qk
        # -> relu (scalar) -> weighted accum (DVE); top-k of strip 0 overlaps
        # strip 1's pipeline on spare DVE slots.
        cidx = const.tile([128, C], U32)
        nc.gpsimd.iota(cidx, pattern=[[1, C]], base=0, channel_multiplier=0,
                       allow_small_or_imprecise_dtypes=True)
        maskhi = const.tile([128, 1], U32)
        nc.vector.memset(maskhi, 0xFFFFFE00)
        mask511 = const.tile([128, 1], U32)
        nc.vector.memset(mask511, 511)
        acc = [const.tile([128, widths[tt]], F32, tag=f"acc{tt}",
                          name=f"acc{tt}") for tt in range(2)]

        def emit_all():
            qroT = [const.tile([128, H, 128], F32, tag=f"qroT{t}",
                               name=f"qroT{t}") for t in range(2)]
            with tc.tile_pool(name="wqp", bufs=12) as wqp, \
                 tc.tile_pool(name="qwork", bufs=3) as qwork, \
                 tc.tile_pool(name="rlp", bufs=3) as rlp, \
                 tc.tile_pool(name="qps", bufs=1, space="PSUM") as qps, \
                 tc.tile_pool(name="tps2", bufs=2, space="PSUM") as tps2, \
                 tc.tile_pool(name="qkps", bufs=2, space="PSUM") as qkps:
                for mg in range(8):          # 8 groups x 2 m-chunks x 2 strips
                    psq = [[qps.tile([128, 512], F32, tag=f"q{tt}{i}",
                                     name=f"psq{tt}{i}") for i in range(2)]
                           for tt in range(2)]
                    for kc in range(12):
                        wtiles = []
                        for i in range(2):
                            mc = 2 * mg + i
                            wt = wqp.tile([128, 512], F32R, tag="wq")
                            nc.sync.dma_start(
                                out=wt,
                                in_=wqT[kc][:, mc * 512:(mc + 1) * 512])
                            wtiles.append(wt)
                        for tt in range(2):
                            lhs = qrT[:, kc, tt * 128:(tt + 1) * 128]
                            for i in range(2):
                                nc.tensor.matmul(psq[tt][i], lhs, wtiles[i],
                                                 start=(kc == 0),
                                                 stop=(kc == 11))
                    for tt in range(2):
                        Wt = widths[tt]
                        for i in range(2):
                            mc = 2 * mg + i
                            q_sb = qwork.tile([128, 4, 128], F32, tag="q_sb")
                            nc.any.tensor_copy(q_sb, psq[tt][i].rearrange(
                                "p (h x) -> p h x", x=128))
                            co = cs_sb[tt][:, 0:64].unsqueeze(1).to_broadcast(
                                [128, 4, 64])
                            si = cs_sb[tt][:, 64:128].unsqueeze(1).to_broadcast(
                                [128, 4, 64])
                            x1 = q_sb[:, :, 0:64]
                            x2 = q_sb[:, :, 64:128]
                            qro = qwork.tile([128, 4, 128], F32, tag="qro")
                            tmp = qwork.tile([128, 4, 128], F32, tag="qtmp")
                            nc.vector.tensor_mul(qro[:, :, 0:64], x1, co)
                            nc.vector.tensor_mul(tmp[:, :, 0:64], x2, si)
                            nc.vector.tensor_sub(qro[:, :, 0:64],
                                                 qro[:, :, 0:64],
                                                 tmp[:, :, 0:64])
                            nc.vector.tensor_mul(qro[:, :, 64:128], x2, co)
                            nc.vector.tensor_mul(tmp[:, :, 64:128], x1, si)
                            nc.vector.tensor_add(qro[:, :, 64:128],
                                                 qro[:, :, 64:128],
                                                 tmp[:, :, 64:128])
                            pst = tps2.tile([128, 4, 128], F32, tag="tp4")
                            for hh in range(4):
                                tp(pst[:, hh, :], qro[:, hh, :])
                            nc.any.tensor_copy(
                                qroT[tt][:, 4 * mc:4 * mc + 4, :], pst)
                            for hh in range(4):
                                h = 4 * mc + hh
                                ps_qk = qkps.tile([128, Wt], F32, tag="qk",
                                                  name=f"qk{tt}")
                                nc.tensor.matmul(ps_qk, qroT[tt][:, h, :],
                                                 kT[:, :Wt], start=True,
                                                 stop=True)
                                rl = rlp.tile([128, Wt], F32, tag="rl",
                                              name=f"rl{tt}")
                                nc.scalar.activation(
                                    rl, ps_qk,
                                    mybir.ActivationFunctionType.Relu)
                                wcol = wts_sb[tt][:, h:h + 1]
                                if h == 0:
                                    nc.vector.tensor_scalar(
                                        acc[tt], rl, wcol, None,
                                        op0=mybir.AluOpType.mult)
                                else:
                                    nc.vector.scalar_tensor_tensor(
                                        out=acc[tt], in0=rl, scalar=wcol,
                                        in1=acc[tt],
                                        op0=mybir.AluOpType.mult,
                                        op1=mybir.AluOpType.add)

        def emit_topk(tt):
            Wt = widths[tt]
            iters = ITERS[tt]
            nk = 8 * iters
            # causal mask: c >= num_comp -> NEG
            cmp = work.tile([128, C], F32, tag="cmp", name="cmp")
            nc.vector.tensor_scalar(cmp[:, :Wt], c4f[:, :Wt], pos_sb[tt], None,
                                    op0=mybir.AluOpType.add)
            mbit = work.tile([128, C], U32, tag="mbit", name="mbit")
            nc.vector.tensor_scalar(mbit[:, :Wt], cmp[:, :Wt], 0.0, None,
                                    op0=mybir.AluOpType.is_lt)
            nc.vector.copy_predicated(acc[tt][:, :Wt], mbit[:, :Wt],
                                      negs[:, :Wt])
            # pack candidate index into the low 9 mantissa bits so one
            # max8+match_replace pass yields value AND index
            accu = acc[tt][:, :Wt].bitcast(U32)
            nc.vector.scalar_tensor_tensor(
                out=accu, in0=accu, scalar=maskhi, in1=cidx[:, :Wt],
                op0=mybir.AluOpType.bitwise_and,
                op1=mybir.AluOpType.bitwise_or)

            idx = tk.tile([128, TOPK], U32, tag="idx", name="idx")
            vals = acc[tt]
            for it in range(iters):
                mx = tk.tile([128, 8], F32, tag="mx", name="mx")
                nc.vector.max(out=mx, in_=vals[:, :Wt])
                nc.vector.tensor_scalar(idx[:, it * 8:(it + 1) * 8],
                                        mx.bitcast(U32), mask511, None,
                                        op0=mybir.AluOpType.bitwise_and)
                nc.vector.match_replace(out=vals[:, :Wt], in_to_replace=mx,
                                        in_values=vals[:, :Wt], imm_value=NEG)

            idx32 = tk.tile([128, TOPK], I32, tag="idx32", name="idx32")
            if nk < TOPK:
                nc.vector.memset(idx32[:, nk:], -1)
            nc.vector.tensor_copy(idx32[:, :nk], idx[:, :nk])
            rmp = work.tile([128, TOPK], F32, tag="rmp", name="rmp")
            nc.vector.tensor_scalar(rmp[:, :nk], c4f[:, :nk], pos_sb[tt], None,
                                    op0=mybir.AluOpType.add)
            rbit = work.tile([128, TOPK], U32, tag="rbit", name="rbit")
            nc.vector.tensor_scalar(rbit[:, :nk], rmp[:, :nk], 0.0, None,
                                    op0=mybir.AluOpType.is_lt)
            nc.vector.copy_predicated(idx32[:, :nk], rbit[:, :nk],
                                      neg1[:, :nk])
            nc.sync.dma_start(out=out_idx[tt * 128:(tt + 1) * 128, :],
                              in_=idx32)

        emit_all()
        emit_topk(0)
        emit_topk(1)

    nc.finalize()
    return nc


def _get(name, *args):
    key = (name, args)
    if key not in _cache:
        _cache[key] = _build_l1() if name == "l1" else _build_l2(*args)
    return _cache[key]


def kernel(hidden_states, qr, positions, W_fused, Wq, Wproj, ape, rms_weight,
           cos_sin_cache, _timing=None):
    hidden_states = np.asarray(hidden_states, np.float32)
    qr = np.asarray(qr, np.float32)
    positions = np.asarray(positions, np.int32)
    W_fused = np.asarray(W_fused, np.float32)
    Wq = np.asarray(Wq, np.float32)
    Wproj = np.asarray(Wproj, np.float32)
    ape = np.asarray(ape, np.float32)
    rms_weight = np.asarray(rms_weight, np.float32)
    cos_sin_cache = np.asarray(cos_sin_cache, np.float32)

    wcomb = np.ascontiguousarray(
        np.concatenate([W_fused.T, Wproj.T], axis=1))          # [7168, 576]
    wqT = np.ascontiguousarray(
        Wq.T.reshape(12, 128, H * D))                          # [kc, qr, m]

    cores = list(range(NC))
    trace = _timing is not None

    in1 = []
    for i in cores:
        rows = []
        for j in PAIRS[i]:
            lo = 128 * j - 4
            if lo < 0:
                blk = np.zeros((132, HID), np.float32)
                blk[4:] = hidden_states[0:128 * j + 128]
            else:
                blk = hidden_states[lo:128 * j + 128]
            rows.append(blk)
        hidden_halo = np.ascontiguousarray(np.concatenate(rows, axis=0))
        cs_rows = np.concatenate(
            [cos_sin_cache[32 * j:32 * j + 32] for j in PAIRS[i]], axis=0)
        hfl = np.array([NEG if j == 0 else 0.0 for j in PAIRS[i]], np.float32)
        in1.append({
            "hidden": hidden_halo, "wcomb": wcomb, "ape": ape,
            "rmsw": rms_weight, "cs_k": np.ascontiguousarray(cs_rows),
            "haloflag": hfl,
        })
    r1 = run_bass_kernel_spmd(_get("l1"), in1, cores, trace=trace,
                              trace_cores=cores if trace else None)

    k_full = np.zeros((C, D), np.float32)
    wts = {}
    for i in cores:
        kl = r1.results[i]["k_loc"]
        for s, j in enumerate(PAIRS[i]):
            k_full[32 * j:32 * j + 32] = kl[32 * s:32 * s + 32]
        wts[i] = r1.results[i]["wts_own"]

    # one SPMD program for all cores: tile A of core i covers tokens
    # 128i..128i+127 (causal width <= 256), tile B covers the mirror tile
    # (width <= 512); masked-NEG columns make the uniform widths correct.
    in2 = []
    for i in cores:
        sel = np.concatenate(
            [np.arange(128 * j, 128 * j + 128) for j in PAIRS[i]])
        in2.append({
            "qr_sh": np.ascontiguousarray(qr[sel]),
            "wqT": wqT,
            "cs_own": np.ascontiguousarray(cos_sin_cache[positions[sel]]),
            "k_full": k_full,
            "wts_own": wts[i],
            "posm3": (positions[sel] - 3).astype(np.float32),
        })
    r2 = run_bass_kernel_spmd(_get("l2", (256, 512)), in2, cores, trace=trace,
                              trace_cores=cores if trace else None)

    out = np.empty((T, TOPK), np.int32)
    for i in cores:
        oi = r2.results[i]["out_idx"]
        for s, j in enumerate(PAIRS[i]):
            out[128 * j:128 * j + 128] = oi[128 * s:128 * s + 128]

    if _timing is not None:
        _timing["l1"] = r1
        _timing["l2"] = r2
    return out


# revision 5
# speedup vs baseline: 1.1332x; 1.0004x over previous
"""DeepseekV4 indexer kernel for 8 trn2 NeuronCores (Bass/Tile) — v2.

vs baseline:
  - q GEMM runs in float32r (11-bit mantissa, 1 cyc/row at N>=512) with the
    token-strip as the stationary operand: 384 N=512 matmuls + 144 LDWs
    instead of 768 N=256 fp32 matmuls (4 cyc/row) + 768 LDWs.
    End-to-end index mismatch from this is 1.36e-2 (deterministic), within
    the 2e-2 gate.  All other GEMMs stay fp32.
  - q comes out token-major [t, h*128+d]; RoPE is done in that layout with
    strided column views (no partition-swap DMAs), then per-head PE
    transposes produce qroT [d, t] for the fp32 qk matmuls.
  - qk / relu-accum / top-k use exact causal widths: token tile j only ever
    scores against compressed keys c < 32*(j+1), and needs only
    min(4*(j+1), 32) top-8 extraction iterations.
  - relu moved to the Scalar engine; the weighted accumulation over heads is
    a single DVE scalar_tensor_tensor per head.
"""
import sys
sys.path.insert(0, '/opt/trn_rl_repo')

from contextlib import ExitStack

import numpy as np

import concourse.bass as bass
import concourse.bacc as bacc
import concourse.tile as tile
from concourse import mybir
from concourse.bass_utils import run_bass_kernel_spmd
from concourse.masks import make_identity

T, HID, QR_DIM, H, D, TOPK, R = 2048, 7168, 1536, 64, 128, 256, 4
C = T // R
NC = 8
EPS = 1e-6
F32 = mybir.dt.float32
F32R = mybir.dt.float32r
I32 = mybir.dt.int32
U32 = mybir.dt.uint32
WTS_SCALE = float(H ** -0.5) * float(D ** -0.5)  # folds q's D**-0.5 into wts
NEG = -1e30

PAIRS = [(i, 15 - i) for i in range(NC)]  # token tiles owned by core i

_cache = {}


# --------------------------------------------------------------------------
# launch 1: compressor -> per-core compressed K (64 rows) + head weights
# (unchanged from baseline except cosmetics)
# --------------------------------------------------------------------------
def _build_l1():
    nc = bacc.Bacc()
    hidden = nc.declare_dram_parameter("hidden", [264, HID], F32, isOutput=False)
    wcomb = nc.declare_dram_parameter("wcomb", [HID, 576], F32, isOutput=False)
    ape = nc.declare_dram_parameter("ape", [8, D], F32, isOutput=False)
    rmsw = nc.declare_dram_parameter("rmsw", [D], F32, isOutput=False)
    cs_k = nc.declare_dram_parameter("cs_k", [64, D], F32, isOutput=False)
    haloflag = nc.declare_dram_parameter("haloflag", [2], F32, isOutput=False)
    k_loc = nc.declare_dram_parameter("k_loc", [64, D], F32, isOutput=True)
    wts_own = nc.declare_dram_parameter("wts_own", [256, H], F32, isOutput=True)

    with tile.TileContext(nc) as tc, ExitStack() as ctx:
        const = ctx.enter_context(tc.tile_pool(name="const", bufs=1))
        big = ctx.enter_context(tc.tile_pool(name="big", bufs=1))
        work = ctx.enter_context(tc.tile_pool(name="work", bufs=2))

        ident = const.tile([128, 128], F32)
        make_identity(nc, ident)

        def tp(ps_out, in_sb):
            p = in_sb.shape[0]
            nc.tensor.transpose(ps_out, in_sb, ident[:p, :p])

        # ---- interleaved: per 512-col group, DMA hidden chunk ->
        # transpose -> 4 GEMM k-chunks (PE starts after the first 0.8 MB) ----
        # ---- fused GEMM: kv_scoreT [4x128, 264] + wtsT [64, 264] ----
        kvt = []
        wts_sb = work.tile([64, 264], F32, tag="wts_sb")
        with tc.tile_pool(name="stg", bufs=3) as stg, \
             tc.tile_pool(name="hidp", bufs=2) as hidp, \
             tc.tile_pool(name="tpsA", bufs=2, space="PSUM") as tpsA, \
             tc.tile_pool(name="wstg", bufs=3) as wstg, \
             tc.tile_pool(name="gps", bufs=1, space="PSUM") as gps:
            kvps = [gps.tile([128, 264], F32, tag=f"kvps{m}", name=f"kvps{m}") for m in range(4)]
            wtsps = gps.tile([64, 264], F32, tag="wtsps")
            for kg in range(14):
                hidT = hidp.tile([128, 4, 264], F32, tag="hidT")
                for (si, (t0, rows)) in enumerate([(0, 128), (128, 128), (256, 8)]):
                    stage = stg.tile([128, 512], F32, tag=f"st{si}",
                                     name=f"st{si}")
                    nc.sync.dma_start(
                        out=stage[:rows, :],
                        in_=hidden[t0:t0 + rows, kg * 512:(kg + 1) * 512])
                    ps = tpsA.tile([128, 512], F32, tag="tp")
                    for u in range(4):
                        tp(ps[:, u * 128:u * 128 + rows],
                           stage[:rows, u * 128:(u + 1) * 128])
                    sv = ps.rearrange("p (u x) -> p u x", x=128)[:, :, :rows]
                    nc.scalar.copy(hidT[:, :, t0:t0 + rows], sv)
                for u in range(4):
                    kc = kg * 4 + u
                    wt = wstg.tile([128, 576], F32, tag="wcomb")
                    nc.sync.dma_start(out=wt, in_=wcomb[kc * 128:(kc + 1) * 128, :])
                    for m in range(4):
                        nc.tensor.matmul(kvps[m], wt[:, m * 128:(m + 1) * 128],
                                         hidT[:, u, :], start=(kc == 0),
                                         stop=(kc == 55))
                    nc.tensor.matmul(wtsps, wt[:, 512:576], hidT[:, u, :],
                                     start=(kc == 0), stop=(kc == 55))
            for m in range(4):
                t = work.tile([128, 264], F32, tag=f"kvt{m}")
                nc.scalar.copy(t, kvps[m])
                kvt.append(t)
            nc.scalar.mul(wts_sb, wtsps, WTS_SCALE)
        kv_old, kv_new, sc_old, sc_new = kvt

        with tc.tile_pool(name="tpsB", bufs=2, space="PSUM") as tpsB:
            # wts -> [t, h] and out
            for s in range(2):
                ps = tpsB.tile([128, 64], F32, tag="wtp")
                tp(ps, wts_sb[:, 4 + 132 * s:132 + 132 * s])
                ob = work.tile([128, 64], F32, tag="wob")
                nc.scalar.copy(ob, ps)
                nc.sync.dma_start(out=wts_own[128 * s:128 * (s + 1), :], in_=ob)

            # ape transposed + replicated [128, 32, 8]
            ape_st = work.tile([8, D], F32, tag="ape_st")
            nc.sync.dma_start(out=ape_st, in_=ape[:])
            aps = tpsB.tile([128, 8], F32, tag="apetp")
            tp(aps, ape_st)
            apeT = const.tile([128, 8], F32)
            nc.scalar.copy(apeT, aps)
            ape_rep = const.tile([128, 32, 8], F32)
            for g in range(32):
                nc.vector.tensor_copy(ape_rep[:, g, :], apeT)

            # rms weight replicated [32, 128]
            rms_rep = const.tile([32, D], F32)
            nc.sync.dma_start(out=rms_rep, in_=bass.AP(
                tensor=rmsw, offset=0, ap=[[0, 32], [1, D]]))

            cs_st = []
            for s in range(2):
                cst = const.tile([32, D], F32, tag=f"cs{s}", name=f"cs{s}")
                nc.sync.dma_start(out=cst, in_=cs_k[32 * s:32 * s + 32, :])
                cs_st.append(cst)

            hf = []
            for s in range(2):
                h = const.tile([128, 1], F32, tag=f"hf{s}")
                nc.sync.dma_start(out=h, in_=bass.AP(
                    tensor=haloflag, offset=s, ap=[[0, 128], [1, 1]]))
                hf.append(h)

            for s in range(2):
                o = 132 * s
                gates = work.tile([128, 32, 8], F32, tag="gates")
                so_v = sc_old[:, o:o + 128].rearrange("p (g x) -> p g x", x=4)
                sn_v = sc_new[:, o + 4:o + 132].rearrange("p (g x) -> p g x", x=4)
                ko_v = kv_old[:, o:o + 128].rearrange("p (g x) -> p g x", x=4)
                kn_v = kv_new[:, o + 4:o + 132].rearrange("p (g x) -> p g x", x=4)
                nc.vector.tensor_add(gates[:, :, 0:4], so_v, ape_rep[:, :, 0:4])
                nc.vector.tensor_add(gates[:, :, 4:8], sn_v, ape_rep[:, :, 4:8])
                # first group's old slots += -1e30 when strip starts at t=0
                nc.vector.tensor_scalar(gates[:, 0, 0:4], gates[:, 0, 0:4],
                                        hf[s], None, op0=mybir.AluOpType.add)
                gmax = work.tile([128, 32], F32, tag="gmax")
                nc.vector.reduce_max(gmax, gates, axis=mybir.AxisListType.X)
                nc.vector.tensor_sub(gates, gates,
                                     gmax.to_broadcast([128, 32, 8]))
                ex = work.tile([128, 32, 8], F32, tag="ex")
                nc.scalar.activation(ex, gates, mybir.ActivationFunctionType.Exp)
                den = work.tile([128, 32], F32, tag="den")
                nc.vector.reduce_sum(den, ex, axis=mybir.AxisListType.X)
                rec = work.tile([128, 32], F32, tag="rec")
                nc.vector.reciprocal(rec, den)
                w8 = work.tile([128, 32, 8], F32, tag="w8")
                nc.vector.tensor_mul(w8, ex, rec.to_broadcast([128, 32, 8]))
                prod = work.tile([128, 32, 8], F32, tag="prod")
                nc.vector.tensor_mul(prod[:, :, 0:4], w8[:, :, 0:4], ko_v)
                nc.vector.tensor_mul(prod[:, :, 4:8], w8[:, :, 4:8], kn_v)
                comp = work.tile([128, 32], F32, tag="comp")
                nc.vector.reduce_sum(comp, prod, axis=mybir.AxisListType.X)

                cps = tpsB.tile([32, 128], F32, tag="ctp")
                tp(cps, comp)
                compT = work.tile([32, D], F32, tag="compT")
                nc.scalar.copy(compT, cps)

                # RMSNorm over d
                sq = work.tile([32, D], F32, tag="sq")
                nc.vector.tensor_mul(sq, compT, compT)
                ssum = work.tile([32, 1], F32, tag="ssum")
                nc.vector.reduce_sum(ssum, sq, axis=mybir.AxisListType.X)
                nc.vector.tensor_scalar(ssum, ssum, 1.0 / D, EPS,
                                        op0=mybir.AluOpType.mult,
                                        op1=mybir.AluOpType.add)
                rt = work.tile([32, 1], F32, tag="rt")
                nc.scalar.sqrt(rt, ssum)
                rs = work.tile([32, 1], F32, tag="rs")
                nc.vector.reciprocal(rs, rt)
                nc.vector.tensor_scalar(compT, compT, rs, None,
                                        op0=mybir.AluOpType.mult)
                nc.vector.tensor_mul(compT, compT, rms_rep)

                # RoPE at compressed positions (all tiles at base partition 0)
                co = cs_st[s][:, 0:64]
                si = cs_st[s][:, 64:128]
                x1 = compT[:, 0:64]
                x2 = compT[:, 64:128]
                tmp = work.tile([32, D], F32, tag="ktmp")
                kx = work.tile([32, D], F32, tag="kx")
                nc.vector.tensor_mul(kx[:, 0:64], x1, co)
                nc.vector.tensor_mul(tmp[:, 0:64], x2, si)
                nc.vector.tensor_sub(kx[:, 0:64], kx[:, 0:64], tmp[:, 0:64])
                nc.vector.tensor_mul(kx[:, 64:128], x2, co)
                nc.vector.tensor_mul(tmp[:, 64:128], x1, si)
                nc.vector.tensor_add(kx[:, 64:128], kx[:, 64:128],
                                     tmp[:, 64:128])
                nc.sync.dma_start(out=k_loc[32 * s:32 * s + 32, :], in_=kx)

    nc.finalize()
    return nc


# --------------------------------------------------------------------------
# launch 2: q GEMM (f32r, token-stationary) + RoPE + qk + scores + top-k
# --------------------------------------------------------------------------
def _build_l2(widths):
    """widths: (W_A, W_B) causal widths of this core's two token tiles."""
    nc = bacc.Bacc()
    qr_sh = nc.declare_dram_parameter("qr_sh", [256, QR_DIM], F32, isOutput=False)
    # Wq.T reshaped [12 kc, 128 qr, 8192 m] and rounded to f32r on device read
    wqT = nc.declare_dram_parameter("wqT", [12, 128, H * D], F32R, isOutput=False)
    cs_own = nc.declare_dram_parameter("cs_own", [256, D], F32, isOutput=False)
    k_full = nc.declare_dram_parameter("k_full", [C, D], F32, isOutput=False)
    wts_own = nc.declare_dram_parameter("wts_own", [256, H], F32, isOutput=False)
    posm3 = nc.declare_dram_parameter("posm3", [256], F32, isOutput=False)
    out_idx = nc.declare_dram_parameter("out_idx", [256, TOPK], I32, isOutput=True)

    ITERS = tuple(min((w // 32) * 4, 32) for w in widths)

    with tile.TileContext(nc) as tc, ExitStack() as ctx:
        const = ctx.enter_context(tc.tile_pool(name="const", bufs=1))
        work = ctx.enter_context(tc.tile_pool(name="work", bufs=2))
        tk = ctx.enter_context(tc.tile_pool(name="tk", bufs=2))

        ident = const.tile([128, 128], F32)
        make_identity(nc, ident)

        def tp(ps_out, in_sb):
            p = in_sb.shape[0]
            nc.tensor.transpose(ps_out, in_sb, ident[:p, :p])

        # ---- prep: qrT (f32r), kT, cos/sin strips, wts, pos ----
        qrT = const.tile([128, 12, 256], F32R)
        kT = const.tile([128, C], F32)
        with tc.tile_pool(name="stg", bufs=2) as stg, \
             tc.tile_pool(name="tps", bufs=2, space="PSUM") as tps:
            for tt in range(2):
                stage = stg.tile([128, QR_DIM], F32, tag="qstage")
                nc.sync.dma_start(out=stage,
                                  in_=qr_sh[tt * 128:(tt + 1) * 128, :])
                for kg in range(3):
                    ps = tps.tile([128, 512], F32, tag="tp")
                    for u in range(4):
                        kc = kg * 4 + u
                        tp(ps[:, u * 128:(u + 1) * 128],
                           stage[:, kc * 128:(kc + 1) * 128])
                    nc.scalar.copy(
                        qrT[:, kg * 4:kg * 4 + 4, tt * 128:(tt + 1) * 128],
                        ps.rearrange("p (u x) -> p u x", x=128))
            kstage = const.tile([128, 4, D], F32)
            nc.sync.dma_start(out=kstage,
                              in_=k_full[:].rearrange("(a p) d -> p a d", p=128))
            for a in range(4):
                ps = tps.tile([128, 512], F32, tag="tp")
                tp(ps[:, :128], kstage[:, a, :])
                nc.scalar.copy(kT[:, a * 128:(a + 1) * 128], ps[:, :128])

        cs_sb, wts_sb, pos_sb = [], [], []
        for tt in range(2):
            csb = const.tile([128, D], F32, tag=f"cs{tt}", name=f"cs{tt}")
            nc.sync.dma_start(out=csb, in_=cs_own[tt * 128:(tt + 1) * 128, :])
            cs_sb.append(csb)
            w = const.tile([128, H], F32, tag=f"wts{tt}", name=f"wts{tt}")
            nc.sync.dma_start(out=w, in_=wts_own[tt * 128:(tt + 1) * 128, :])
            wts_sb.append(w)
            p = const.tile([128, 1], F32, tag=f"pos{tt}", name=f"pos{tt}")
            nc.sync.dma_start(out=p, in_=posm3[tt * 128:(tt + 1) * 128])
            pos_sb.append(p)

        c4p = const.tile([128, C], F32)
        nc.gpsimd.iota(c4p, pattern=[[4, C]], base=0, channel_multiplier=0,
                       allow_small_or_imprecise_dtypes=True)
        c4f = const.tile([128, C], F32)
        nc.vector.tensor_scalar(c4f, c4p, -1.0, None,
                                op0=mybir.AluOpType.mult)
        negs = const.tile([128, C], F32)
        nc.vector.memset(negs, NEG)
        neg1 = const.tile([128, TOPK], I32)
        nc.vector.memset(neg1, -1)

        # ---- per-strip pipeline: q GEMM (f32r) -> RoPE -> transpose -> qk
        # -> relu (scalar) -> weighted accum (DVE); top-k of strip 0 overlaps
        # strip 1's pipeline on spare DVE slots.
        cidx = const.tile([128, C], U32)
        nc.gpsimd.iota(cidx, pattern=[[1, C]], base=0, channel_multiplier=0,
                       allow_small_or_imprecise_dtypes=True)
        maskhi = const.tile([128, 1], U32)
        nc.vector.memset(maskhi, 0xFFFFFE00)
        mask511 = const.tile([128, 1], U32)
        nc.vector.memset(mask511, 511)
        acc = [const.tile([128, widths[tt]], F32, tag=f"acc{tt}",
                          name=f"acc{tt}") for tt in range(2)]

        def emit_all():
            qroT = [const.tile([128, H, 128], F32, tag=f"qroT{t}",
                               name=f"qroT{t}") for t in range(2)]
            with tc.tile_pool(name="wqp", bufs=12) as wqp, \
                 tc.tile_pool(name="qwork", bufs=3) as qwork, \
                 tc.tile_pool(name="rlp", bufs=3) as rlp, \
                 tc.tile_pool(name="qps", bufs=1, space="PSUM") as qps, \
                 tc.tile_pool(name="tps2", bufs=2, space="PSUM") as tps2, \
                 tc.tile_pool(name="qkps", bufs=2, space="PSUM") as qkps:
                for mc in range(16):         # 1 m-chunk x 2 strips per group
                    psq = [qps.tile([128, 512], F32, tag=f"q{tt}",
                                    name=f"psq{tt}") for tt in range(2)]
                    for kc in range(12):
                        wt = wqp.tile([128, 512], F32R, tag="wq")
                        nc.sync.dma_start(
                            out=wt, in_=wqT[kc][:, mc * 512:(mc + 1) * 512])
                        for tt in range(2):
                            lhs = qrT[:, kc, tt * 128:(tt + 1) * 128]
                            nc.tensor.matmul(psq[tt], lhs, wt,
                                             start=(kc == 0), stop=(kc == 11))
                    for tt in range(2):
                        Wt = widths[tt]
                        if True:
                            q_sb = qwork.tile([128, 4, 128], F32, tag="q_sb")
                            nc.any.tensor_copy(q_sb, psq[tt].rearrange(
                                "p (h x) -> p h x", x=128))
                            co = cs_sb[tt][:, 0:64].unsqueeze(1).to_broadcast(
                                [128, 4, 64])
                            si = cs_sb[tt][:, 64:128].unsqueeze(1).to_broadcast(
                                [128, 4, 64])
                            x1 = q_sb[:, :, 0:64]
                            x2 = q_sb[:, :, 64:128]
                            qro = qwork.tile([128, 4, 128], F32, tag="qro")
                            tmp = qwork.tile([128, 4, 128], F32, tag="qtmp")
                            nc.vector.tensor_mul(qro[:, :, 0:64], x1, co)
                            nc.vector.tensor_mul(tmp[:, :, 0:64], x2, si)
                            nc.vector.tensor_sub(qro[:, :, 0:64],
                                                 qro[:, :, 0:64],
                                                 tmp[:, :, 0:64])
                            nc.vector.tensor_mul(qro[:, :, 64:128], x2, co)
                            nc.vector.tensor_mul(tmp[:, :, 64:128], x1, si)
                            nc.vector.tensor_add(qro[:, :, 64:128],
                                                 qro[:, :, 64:128],
                                                 tmp[:, :, 64:128])
                            pst = tps2.tile([128, 4, 128], F32, tag="tp4")
                            for hh in range(4):
                                tp(pst[:, hh, :], qro[:, hh, :])
                            nc.any.tensor_copy(
                                qroT[tt][:, 4 * mc:4 * mc + 4, :], pst)
                            for hp in range(2):
                                ps_qk = qkps.tile([128, 2, 512], F32,
                                                  tag="qk", name=f"qk{tt}")
                                for z in range(2):
                                    h = 4 * mc + 2 * hp + z
                                    nc.tensor.matmul(
                                        ps_qk[:, z, :Wt], qroT[tt][:, h, :],
                                        kT[:, :Wt], start=True, stop=True)
                                rl = rlp.tile([128, 2, Wt], F32, tag="rl",
                                              name=f"rl{tt}")
                                nc.scalar.activation(
                                    rl, ps_qk[:, :, :Wt],
                                    mybir.ActivationFunctionType.Relu)
                                for z in range(2):
                                    h = 4 * mc + 2 * hp + z
                                    wcol = wts_sb[tt][:, h:h + 1]
                                    if h == 0:
                                        nc.vector.tensor_scalar(
                                            acc[tt], rl[:, z, :], wcol, None,
                                            op0=mybir.AluOpType.mult)
                                    else:
                                        nc.vector.scalar_tensor_tensor(
                                            out=acc[tt], in0=rl[:, z, :],
                                            scalar=wcol, in1=acc[tt],
                                            op0=mybir.AluOpType.mult,
                                            op1=mybir.AluOpType.add)

        def emit_topk(tt):
            Wt = widths[tt]
            iters = ITERS[tt]
            nk = 8 * iters
            # causal mask: c >= num_comp -> NEG
            cmp = work.tile([128, C], F32, tag="cmp", name="cmp")
            nc.vector.tensor_scalar(cmp[:, :Wt], c4f[:, :Wt], pos_sb[tt], None,
                                    op0=mybir.AluOpType.add)
            mbit = work.tile([128, C], U32, tag="mbit", name="mbit")
            nc.vector.tensor_scalar(mbit[:, :Wt], cmp[:, :Wt], 0.0, None,
                                    op0=mybir.AluOpType.is_lt)
            nc.vector.copy_predicated(acc[tt][:, :Wt], mbit[:, :Wt],
                                      negs[:, :Wt])
            # pack candidate index into the low 9 mantissa bits so one
            # max8+match_replace pass yields value AND index
            accu = acc[tt][:, :Wt].bitcast(U32)
            nc.vector.scalar_tensor_tensor(
                out=accu, in0=accu, scalar=maskhi, in1=cidx[:, :Wt],
                op0=mybir.AluOpType.bitwise_and,
                op1=mybir.AluOpType.bitwise_or)

            idx = tk.tile([128, TOPK], U32, tag="idx", name="idx")
            vals = acc[tt]
            for it in range(iters):
                mx = tk.tile([128, 8], F32, tag="mx", name="mx")
                nc.vector.max(out=mx, in_=vals[:, :Wt])
                nc.vector.tensor_scalar(idx[:, it * 8:(it + 1) * 8],
                                        mx.bitcast(U32), mask511, None,
                                        op0=mybir.AluOpType.bitwise_and)
                nc.vector.match_replace(out=vals[:, :Wt], in_to_replace=mx,
                                        in_values=vals[:, :Wt], imm_value=NEG)

            idx32 = tk.tile([128, TOPK], I32, tag="idx32", name="idx32")
            if nk < TOPK:
                nc.vector.memset(idx32[:, nk:], -1)
            nc.vector.tensor_copy(idx32[:, :nk], idx[:, :nk])
            rmp = work.tile([128, TOPK], F32, tag="rmp", name="rmp")
            nc.vector.tensor_scalar(rmp[:, :nk], c4f[:, :nk], pos_sb[tt], None,
                                    op0=mybir.AluOpType.add)
            rbit = work.tile([128, TOPK], U32, tag="rbit", name="rbit")
            nc.vector.tensor_scalar(rbit[:, :nk], rmp[:, :nk], 0.0, None,
                                    op0=mybir.AluOpType.is_lt)
            nc.vector.copy_predicated(idx32[:, :nk], rbit[:, :nk],
                                      neg1[:, :nk])
            nc.sync.dma_start(out=out_idx[tt * 128:(tt + 1) * 128, :],
                              in_=idx32)

        emit_all()
        emit_topk(0)
        emit_topk(1)

    nc.finalize()
    return nc


def _get(name, *args):
    key = (name, args)
    if key not in _cache:
        _cache[key] = _build_l1() if name == "l1" else _build_l2(*args)
    return _cache[key]


def kernel(hidden_states, qr, positions, W_fused, Wq, Wproj, ape, rms_weight,
           cos_sin_cache, _timing=None):
    hidden_states = np.asarray(hidden_states, np.float32)
    qr = np.asarray(qr, np.float32)
    positions = np.asarray(positions, np.int32)
    W_fused = np.asarray(W_fused, np.float32)
    Wq = np.asarray(Wq, np.float32)
    Wproj = np.asarray(Wproj, np.float32)
    ape = np.asarray(ape, np.float32)
    rms_weight = np.asarray(rms_weight, np.float32)
    cos_sin_cache = np.asarray(cos_sin_cache, np.float32)

    wcomb = np.ascontiguousarray(
        np.concatenate([W_fused.T, Wproj.T], axis=1))          # [7168, 576]
    wqT = np.ascontiguousarray(
        Wq.T.reshape(12, 128, H * D))                          # [kc, qr, m]

    cores = list(range(NC))
    trace = _timing is not None

    in1 = []
    for i in cores:
        rows = []
        for j in PAIRS[i]:
            lo = 128 * j - 4
            if lo < 0:
                blk = np.zeros((132, HID), np.float32)
                blk[4:] = hidden_states[0:128 * j + 128]
            else:
                blk = hidden_states[lo:128 * j + 128]
            rows.append(blk)
        hidden_halo = np.ascontiguousarray(np.concatenate(rows, axis=0))
        cs_rows = np.concatenate(
            [cos_sin_cache[32 * j:32 * j + 32] for j in PAIRS[i]], axis=0)
        hfl = np.array([NEG if j == 0 else 0.0 for j in PAIRS[i]], np.float32)
        in1.append({
            "hidden": hidden_halo, "wcomb": wcomb, "ape": ape,
            "rmsw": rms_weight, "cs_k": np.ascontiguousarray(cs_rows),
            "haloflag": hfl,
        })
    r1 = run_bass_kernel_spmd(_get("l1"), in1, cores, trace=trace,
                              trace_cores=cores if trace else None)

    k_full = np.zeros((C, D), np.float32)
    wts = {}
    for i in cores:
        kl = r1.results[i]["k_loc"]
        for s, j in enumerate(PAIRS[i]):
            k_full[32 * j:32 * j + 32] = kl[32 * s:32 * s + 32]
        wts[i] = r1.results[i]["wts_own"]

    # one SPMD program for all cores: tile A of core i covers tokens
    # 128i..128i+127 (causal width <= 256), tile B covers the mirror tile
    # (width <= 512); masked-NEG columns make the uniform widths correct.
    in2 = []
    for i in cores:
        sel = np.concatenate(
            [np.arange(128 * j, 128 * j + 128) for j in PAIRS[i]])
        in2.append({
            "qr_sh": np.ascontiguousarray(qr[sel]),
            "wqT": wqT,
            "cs_own": np.ascontiguousarray(cos_sin_cache[positions[sel]]),
            "k_full": k_full,
            "wts_own": wts[i],
            "posm3": (positions[sel] - 3).astype(np.float32),
        })
    r2 = run_bass_kernel_spmd(_get("l2", (256, 512)), in2, cores, trace=trace,
                              trace_cores=cores if trace else None)

    out = np.empty((T, TOPK), np.int32)
    for i in cores:
        oi = r2.results[i]["out_idx"]
        for s, j in enumerate(PAIRS[i]):
            out[128 * j:128 * j + 128] = oi[128 * s:128 * s + 128]

    if _timing is not None:
        _timing["l1"] = r1
        _timing["l2"] = r2
    return out


# revision 6
# speedup vs baseline: 1.1631x; 1.0264x over previous
"""DeepseekV4 indexer kernel for 8 trn2 NeuronCores (Bass/Tile).

Token-sharded, two bass launches; core i owns token tiles (i, 15-i) so the
causally-pruned qk/top-k work is balanced across cores.

Launch 1 (per core, its 264-token halo'd shard): interleaved per 512-column
group, DMA hidden chunk -> PE-transpose -> fused W_fused|Wproj GEMM (fp32)
-> compressor softmax -> RMSNorm -> RoPE -> 64 rows of compressed K + per-
token head weights.  Host concatenates the per-core K into the full [512,128]
K (the all-gather) between launches.

Launch 2 (per core, 256 tokens): single pass over Wq with the q GEMM in
float32r (11-bit mantissa, 1 cyc/row at N=512) using the token-strip as the
stationary operand (384 N=512 matmuls, LDW-light).  fp32r on the q GEMM
alone gives a deterministic 1.36e-2 index mismatch - well inside the 2e-2
gate; fp32r anywhere else pushes past it, so kv/qk/wts GEMMs stay fp32.
q comes out token-major; RoPE uses strided column views (no partition-swap
DMAs), per-head PE transposes feed the fp32 qk matmuls, relu runs batched on
the Scalar engine (2 heads per bank-aligned 2-bank PSUM tile), and the
weighted head-accumulation is one DVE scalar_tensor_tensor per head.
Top-k packs the candidate index into the low 9 mantissa bits of the score so
each of 32 iterations is just max8 + tiny-AND + match_replace (no
find_index8 pass).  All phases are pipelined per m-chunk so PE / DVE / ACT /
DMA overlap; top-k runs as the tail.

kernel(**inputs) takes the FULL unsharded inputs and returns [2048,256] int32.
"""
import sys
sys.path.insert(0, '/opt/trn_rl_repo')

from contextlib import ExitStack

import numpy as np

import concourse.bass as bass
import concourse.bacc as bacc
import concourse.tile as tile
from concourse import mybir
from concourse.bass_utils import run_bass_kernel_spmd
from concourse.masks import make_identity

T, HID, QR_DIM, H, D, TOPK, R = 2048, 7168, 1536, 64, 128, 256, 4
C = T // R
NC = 8
EPS = 1e-6
F32 = mybir.dt.float32
F32R = mybir.dt.float32r
I32 = mybir.dt.int32
U32 = mybir.dt.uint32
WTS_SCALE = float(H ** -0.5) * float(D ** -0.5)  # folds q's D**-0.5 into wts
NEG = -1e30

PAIRS = [(i, 15 - i) for i in range(NC)]  # token tiles owned by core i

_cache = {}


# --------------------------------------------------------------------------
# launch 1: compressor -> per-core compressed K (64 rows) + head weights
# (unchanged from baseline except cosmetics)
# --------------------------------------------------------------------------
def _build_l1():
    nc = bacc.Bacc()
    hidden = nc.declare_dram_parameter("hidden", [264, HID], F32, isOutput=False)
    wcomb = nc.declare_dram_parameter("wcomb", [HID, 576], F32, isOutput=False)
    ape = nc.declare_dram_parameter("ape", [8, D], F32, isOutput=False)
    rmsw = nc.declare_dram_parameter("rmsw", [D], F32, isOutput=False)
    cs_k = nc.declare_dram_parameter("cs_k", [64, D], F32, isOutput=False)
    haloflag = nc.declare_dram_parameter("haloflag", [2], F32, isOutput=False)
    k_loc = nc.declare_dram_parameter("k_loc", [64, D], F32, isOutput=True)
    wts_own = nc.declare_dram_parameter("wts_own", [256, H], F32, isOutput=True)

    with tile.TileContext(nc) as tc, ExitStack() as ctx:
        const = ctx.enter_context(tc.tile_pool(name="const", bufs=1))
        big = ctx.enter_context(tc.tile_pool(name="big", bufs=1))
        work = ctx.enter_context(tc.tile_pool(name="work", bufs=2))

        ident = const.tile([128, 128], F32)
        make_identity(nc, ident)

        def tp(ps_out, in_sb):
            p = in_sb.shape[0]
            nc.tensor.transpose(ps_out, in_sb, ident[:p, :p])

        # ---- interleaved: per 512-col group, DMA hidden chunk ->
        # transpose -> 4 GEMM k-chunks (PE starts after the first 0.8 MB) ----
        # ---- fused GEMM: kv_scoreT [4x128, 264] + wtsT [64, 264] ----
        kvt = []
        wts_sb = work.tile([64, 264], F32, tag="wts_sb")
        with tc.tile_pool(name="stg", bufs=3) as stg, \
             tc.tile_pool(name="hidp", bufs=2) as hidp, \
             tc.tile_pool(name="tpsA", bufs=2, space="PSUM") as tpsA, \
             tc.tile_pool(name="wstg", bufs=3) as wstg, \
             tc.tile_pool(name="gps", bufs=1, space="PSUM") as gps:
            kvps = [gps.tile([128, 264], F32, tag=f"kvps{m}", name=f"kvps{m}") for m in range(4)]
            wtsps = gps.tile([64, 264], F32, tag="wtsps")
            for kg in range(14):
                hidT = hidp.tile([128, 4, 264], F32, tag="hidT")
                for (si, (t0, rows)) in enumerate([(0, 128), (128, 128), (256, 8)]):
                    stage = stg.tile([128, 512], F32, tag=f"st{si}",
                                     name=f"st{si}")
                    nc.sync.dma_start(
                        out=stage[:rows, :],
                        in_=hidden[t0:t0 + rows, kg * 512:(kg + 1) * 512])
                    ps = tpsA.tile([128, 512], F32, tag="tp")
                    for u in range(4):
                        tp(ps[:, u * 128:u * 128 + rows],
                           stage[:rows, u * 128:(u + 1) * 128])
                    sv = ps.rearrange("p (u x) -> p u x", x=128)[:, :, :rows]
                    nc.scalar.copy(hidT[:, :, t0:t0 + rows], sv)
                for u in range(4):
                    kc = kg * 4 + u
                    wt = wstg.tile([128, 576], F32, tag="wcomb")
                    nc.sync.dma_start(out=wt, in_=wcomb[kc * 128:(kc + 1) * 128, :])
                    for m in range(4):
                        nc.tensor.matmul(kvps[m], wt[:, m * 128:(m + 1) * 128],
                                         hidT[:, u, :], start=(kc == 0),
                                         stop=(kc == 55))
                    nc.tensor.matmul(wtsps, wt[:, 512:576], hidT[:, u, :],
                                     start=(kc == 0), stop=(kc == 55))
            for m in range(4):
                t = work.tile([128, 264], F32, tag=f"kvt{m}")
                nc.scalar.copy(t, kvps[m])
                kvt.append(t)
            nc.scalar.mul(wts_sb, wtsps, WTS_SCALE)
        kv_old, kv_new, sc_old, sc_new = kvt

        with tc.tile_pool(name="tpsB", bufs=2, space="PSUM") as tpsB:
            # wts -> [t, h] and out
            for s in range(2):
                ps = tpsB.tile([128, 64], F32, tag="wtp")
                tp(ps, wts_sb[:, 4 + 132 * s:132 + 132 * s])
                ob = work.tile([128, 64], F32, tag="wob")
                nc.scalar.copy(ob, ps)
                nc.sync.dma_start(out=wts_own[128 * s:128 * (s + 1), :], in_=ob)

            # ape transposed + replicated [128, 32, 8]
            ape_st = work.tile([8, D], F32, tag="ape_st")
            nc.sync.dma_start(out=ape_st, in_=ape[:])
            aps = tpsB.tile([128, 8], F32, tag="apetp")
            tp(aps, ape_st)
            apeT = const.tile([128, 8], F32)
            nc.scalar.copy(apeT, aps)
            ape_rep = const.tile([128, 32, 8], F32)
            for g in range(32):
                nc.vector.tensor_copy(ape_rep[:, g, :], apeT)

            # rms weight replicated [32, 128]
            rms_rep = const.tile([32, D], F32)
            nc.sync.dma_start(out=rms_rep, in_=bass.AP(
                tensor=rmsw, offset=0, ap=[[0, 32], [1, D]]))

            cs_st = []
            for s in range(2):
                cst = const.tile([32, D], F32, tag=f"cs{s}", name=f"cs{s}")
                nc.sync.dma_start(out=cst, in_=cs_k[32 * s:32 * s + 32, :])
                cs_st.append(cst)

            hf = []
            for s in range(2):
                h = const.tile([128, 1], F32, tag=f"hf{s}")
                nc.sync.dma_start(out=h, in_=bass.AP(
                    tensor=haloflag, offset=s, ap=[[0, 128], [1, 1]]))
                hf.append(h)

            for s in range(2):
                o = 132 * s
                gates = work.tile([128, 32, 8], F32, tag="gates")
                so_v = sc_old[:, o:o + 128].rearrange("p (g x) -> p g x", x=4)
                sn_v = sc_new[:, o + 4:o + 132].rearrange("p (g x) -> p g x", x=4)
                ko_v = kv_old[:, o:o + 128].rearrange("p (g x) -> p g x", x=4)
                kn_v = kv_new[:, o + 4:o + 132].rearrange("p (g x) -> p g x", x=4)
                nc.vector.tensor_add(gates[:, :, 0:4], so_v, ape_rep[:, :, 0:4])
                nc.vector.tensor_add(gates[:, :, 4:8], sn_v, ape_rep[:, :, 4:8])
                # first group's old slots += -1e30 when strip starts at t=0
                nc.vector.tensor_scalar(gates[:, 0, 0:4], gates[:, 0, 0:4],
                                        hf[s], None, op0=mybir.AluOpType.add)
                gmax = work.tile([128, 32], F32, tag="gmax")
                nc.vector.reduce_max(gmax, gates, axis=mybir.AxisListType.X)
                nc.vector.tensor_sub(gates, gates,
                                     gmax.to_broadcast([128, 32, 8]))
                ex = work.tile([128, 32, 8], F32, tag="ex")
                nc.scalar.activation(ex, gates, mybir.ActivationFunctionType.Exp)
                den = work.tile([128, 32], F32, tag="den")
                nc.vector.reduce_sum(den, ex, axis=mybir.AxisListType.X)
                rec = work.tile([128, 32], F32, tag="rec")
                nc.vector.reciprocal(rec, den)
                w8 = work.tile([128, 32, 8], F32, tag="w8")
                nc.vector.tensor_mul(w8, ex, rec.to_broadcast([128, 32, 8]))
                prod = work.tile([128, 32, 8], F32, tag="prod")
                nc.vector.tensor_mul(prod[:, :, 0:4], w8[:, :, 0:4], ko_v)
                nc.vector.tensor_mul(prod[:, :, 4:8], w8[:, :, 4:8], kn_v)
                comp = work.tile([128, 32], F32, tag="comp")
                nc.vector.reduce_sum(comp, prod, axis=mybir.AxisListType.X)

                cps = tpsB.tile([32, 128], F32, tag="ctp")
                tp(cps, comp)
                compT = work.tile([32, D], F32, tag="compT")
                nc.scalar.copy(compT, cps)

                # RMSNorm over d
                sq = work.tile([32, D], F32, tag="sq")
                nc.vector.tensor_mul(sq, compT, compT)
                ssum = work.tile([32, 1], F32, tag="ssum")
                nc.vector.reduce_sum(ssum, sq, axis=mybir.AxisListType.X)
                nc.vector.tensor_scalar(ssum, ssum, 1.0 / D, EPS,
                                        op0=mybir.AluOpType.mult,
                                        op1=mybir.AluOpType.add)
                rt = work.tile([32, 1], F32, tag="rt")
                nc.scalar.sqrt(rt, ssum)
                rs = work.tile([32, 1], F32, tag="rs")
                nc.vector.reciprocal(rs, rt)
                nc.vector.tensor_scalar(compT, compT, rs, None,
                                        op0=mybir.AluOpType.mult)
                nc.vector.tensor_mul(compT, compT, rms_rep)

                # RoPE at compressed positions (all tiles at base partition 0)
                co = cs_st[s][:, 0:64]
                si = cs_st[s][:, 64:128]
                x1 = compT[:, 0:64]
                x2 = compT[:, 64:128]
                tmp = work.tile([32, D], F32, tag="ktmp")
                kx = work.tile([32, D], F32, tag="kx")
                nc.vector.tensor_mul(kx[:, 0:64], x1, co)
                nc.vector.tensor_mul(tmp[:, 0:64], x2, si)
                nc.vector.tensor_sub(kx[:, 0:64], kx[:, 0:64], tmp[:, 0:64])
                nc.vector.tensor_mul(kx[:, 64:128], x2, co)
                nc.vector.tensor_mul(tmp[:, 64:128], x1, si)
                nc.vector.tensor_add(kx[:, 64:128], kx[:, 64:128],
                                     tmp[:, 64:128])
                nc.sync.dma_start(out=k_loc[32 * s:32 * s + 32, :], in_=kx)

    nc.finalize()
    return nc


# --------------------------------------------------------------------------
# launch 2: q GEMM (f32r, token-stationary) + RoPE + qk + scores + top-k
# --------------------------------------------------------------------------
def _build_l2(widths):
    """widths: (W_A, W_B) causal widths of this core's two token tiles."""
    nc = bacc.Bacc()
    qr_sh = nc.declare_dram_parameter("qr_sh", [256, QR_DIM], F32, isOutput=False)
    # Wq.T reshaped [12 kc, 128 qr, 8192 m] and rounded to f32r on device read
    wqT = nc.declare_dram_parameter("wqT", [12, 128, H * D], F32R, isOutput=False)
    cs_own = nc.declare_dram_parameter("cs_own", [256, D], F32, isOutput=False)
    k_full = nc.declare_dram_parameter("k_full", [C, D], F32, isOutput=False)
    wts_own = nc.declare_dram_parameter("wts_own", [256, H], F32, isOutput=False)
    posm3 = nc.declare_dram_parameter("posm3", [256], F32, isOutput=False)
    out_idx = nc.declare_dram_parameter("out_idx", [256, TOPK], I32, isOutput=True)

    ITERS = tuple(min((w // 32) * 4, 32) for w in widths)

    with tile.TileContext(nc) as tc, ExitStack() as ctx:
        const = ctx.enter_context(tc.tile_pool(name="const", bufs=1))
        work = ctx.enter_context(tc.tile_pool(name="work", bufs=2))
        tk = ctx.enter_context(tc.tile_pool(name="tk", bufs=2))

        ident = const.tile([128, 128], F32)
        make_identity(nc, ident)

        def tp(ps_out, in_sb):
            p = in_sb.shape[0]
            nc.tensor.transpose(ps_out, in_sb, ident[:p, :p])

        # ---- prep: qrT (f32r), kT, cos/sin strips, wts, pos ----
        qrT = const.tile([128, 12, 256], F32R)
        kT = const.tile([128, C], F32)
        with tc.tile_pool(name="stg", bufs=2) as stg, \
             tc.tile_pool(name="tps", bufs=2, space="PSUM") as tps:
            for tt in range(2):
                stage = stg.tile([128, QR_DIM], F32, tag="qstage")
                nc.sync.dma_start(out=stage,
                                  in_=qr_sh[tt * 128:(tt + 1) * 128, :])
                for kg in range(3):
                    ps = tps.tile([128, 512], F32, tag="tp")
                    for u in range(4):
                        kc = kg * 4 + u
                        tp(ps[:, u * 128:(u + 1) * 128],
                           stage[:, kc * 128:(kc + 1) * 128])
                    nc.scalar.copy(
                        qrT[:, kg * 4:kg * 4 + 4, tt * 128:(tt + 1) * 128],
                        ps.rearrange("p (u x) -> p u x", x=128))
            kstage = const.tile([128, 4, D], F32)
            nc.sync.dma_start(out=kstage,
                              in_=k_full[:].rearrange("(a p) d -> p a d", p=128))
            for a in range(4):
                ps = tps.tile([128, 512], F32, tag="tp")
                tp(ps[:, :128], kstage[:, a, :])
                nc.scalar.copy(kT[:, a * 128:(a + 1) * 128], ps[:, :128])

        cs_sb, wts_sb, pos_sb = [], [], []
        for tt in range(2):
            csb = const.tile([128, D], F32, tag=f"cs{tt}", name=f"cs{tt}")
            nc.sync.dma_start(out=csb, in_=cs_own[tt * 128:(tt + 1) * 128, :])
            cs_sb.append(csb)
            w = const.tile([128, H], F32, tag=f"wts{tt}", name=f"wts{tt}")
            nc.sync.dma_start(out=w, in_=wts_own[tt * 128:(tt + 1) * 128, :])
            wts_sb.append(w)
            p = const.tile([128, 1], F32, tag=f"pos{tt}", name=f"pos{tt}")
            nc.sync.dma_start(out=p, in_=posm3[tt * 128:(tt + 1) * 128])
            pos_sb.append(p)

        c4p = const.tile([128, C], F32)
        nc.gpsimd.iota(c4p, pattern=[[4, C]], base=0, channel_multiplier=0,
                       allow_small_or_imprecise_dtypes=True)
        c4f = const.tile([128, C], F32)
        nc.vector.tensor_scalar(c4f, c4p, -1.0, None,
                                op0=mybir.AluOpType.mult)
        negs = const.tile([128, C], F32)
        nc.vector.memset(negs, NEG)
        neg1 = const.tile([128, TOPK], I32)
        nc.vector.memset(neg1, -1)

        # ---- per-strip pipeline: q GEMM (f32r) -> RoPE -> transpose -> qk
        # -> relu (scalar) -> weighted accum (DVE); top-k of strip 0 overlaps
        # strip 1's pipeline on spare DVE slots.
        cidx = const.tile([128, C], U32)
        nc.gpsimd.iota(cidx, pattern=[[1, C]], base=0, channel_multiplier=0,
                       allow_small_or_imprecise_dtypes=True)
        maskhi = const.tile([128, 1], U32)
        nc.vector.memset(maskhi, 0xFFFFFE00)
        mask511 = const.tile([128, 1], U32)
        nc.vector.memset(mask511, 511)
        acc = [const.tile([128, widths[tt]], F32, tag=f"acc{tt}",
                          name=f"acc{tt}") for tt in range(2)]

        def emit_all():
            qroT = [const.tile([128, H, 128], F32, tag=f"qroT{t}",
                               name=f"qroT{t}") for t in range(2)]
            with tc.tile_pool(name="wqp", bufs=12) as wqp, \
                 tc.tile_pool(name="qwork", bufs=3) as qwork, \
                 tc.tile_pool(name="rlp", bufs=3) as rlp, \
                 tc.tile_pool(name="qps", bufs=1, space="PSUM") as qps, \
                 tc.tile_pool(name="tps2", bufs=2, space="PSUM") as tps2, \
                 tc.tile_pool(name="qkps", bufs=2, space="PSUM") as qkps:
                for mc in range(16):         # 1 m-chunk x 2 strips per group
                    psq = [qps.tile([128, 512], F32, tag=f"q{tt}",
                                    name=f"psq{tt}") for tt in range(2)]
                    for kc in range(12):
                        wt = wqp.tile([128, 512], F32R, tag="wq")
                        nc.sync.dma_start(
                            out=wt, in_=wqT[kc][:, mc * 512:(mc + 1) * 512])
                        for tt in range(2):
                            lhs = qrT[:, kc, tt * 128:(tt + 1) * 128]
                            nc.tensor.matmul(psq[tt], lhs, wt,
                                             start=(kc == 0), stop=(kc == 11))
                    for tt in range(2):
                        Wt = widths[tt]
                        if True:
                            q_sb = qwork.tile([128, 4, 128], F32, tag="q_sb")
                            nc.any.tensor_copy(q_sb, psq[tt].rearrange(
                                "p (h x) -> p h x", x=128))
                            co = cs_sb[tt][:, 0:64].unsqueeze(1).to_broadcast(
                                [128, 4, 64])
                            si = cs_sb[tt][:, 64:128].unsqueeze(1).to_broadcast(
                                [128, 4, 64])
                            x1 = q_sb[:, :, 0:64]
                            x2 = q_sb[:, :, 64:128]
                            qro = qwork.tile([128, 4, 128], F32, tag="qro")
                            tmp = qwork.tile([128, 4, 128], F32, tag="qtmp")
                            nc.vector.tensor_mul(qro[:, :, 0:64], x1, co)
                            nc.vector.tensor_mul(tmp[:, :, 0:64], x2, si)
                            nc.vector.tensor_sub(qro[:, :, 0:64],
                                                 qro[:, :, 0:64],
                                                 tmp[:, :, 0:64])
                            nc.vector.tensor_mul(qro[:, :, 64:128], x2, co)
                            nc.vector.tensor_mul(tmp[:, :, 64:128], x1, si)
                            nc.vector.tensor_add(qro[:, :, 64:128],
                                                 qro[:, :, 64:128],
                                                 tmp[:, :, 64:128])
                            pst = tps2.tile([128, 4, 128], F32, tag="tp4")
                            for hh in range(4):
                                tp(pst[:, hh, :], qro[:, hh, :])
                            nc.any.tensor_copy(
                                qroT[tt][:, 4 * mc:4 * mc + 4, :], pst)
                            for hp in range(2):
                                ps_qk = qkps.tile([128, 2, 512], F32,
                                                  tag="qk", name=f"qk{tt}")
                                for z in range(2):
                                    h = 4 * mc + 2 * hp + z
                                    nc.tensor.matmul(
                                        ps_qk[:, z, :Wt], qroT[tt][:, h, :],
                                        kT[:, :Wt], start=True, stop=True)
                                rl = rlp.tile([128, 2, Wt], F32, tag="rl",
                                              name=f"rl{tt}")
                                nc.scalar.activation(
                                    rl, ps_qk[:, :, :Wt],
                                    mybir.ActivationFunctionType.Relu)
                                for z in range(2):
                                    h = 4 * mc + 2 * hp + z
                                    wcol = wts_sb[tt][:, h:h + 1]
                                    if h == 0:
                                        nc.vector.tensor_scalar(
                                            acc[tt], rl[:, z, :], wcol, None,
                                            op0=mybir.AluOpType.mult)
                                    else:
                                        nc.vector.scalar_tensor_tensor(
                                            out=acc[tt], in0=rl[:, z, :],
                                            scalar=wcol, in1=acc[tt],
                                            op0=mybir.AluOpType.mult,
                                            op1=mybir.AluOpType.add)

        def emit_topk(tt):
            Wt = widths[tt]
            iters = ITERS[tt]
            nk = 8 * iters
            # causal mask: c >= num_comp -> NEG
            cmp = work.tile([128, C], F32, tag="cmp", name="cmp")
            nc.vector.tensor_scalar(cmp[:, :Wt], c4f[:, :Wt], pos_sb[tt], None,
                                    op0=mybir.AluOpType.add)
            mbit = work.tile([128, C], U32, tag="mbit", name="mbit")
            nc.vector.tensor_scalar(mbit[:, :Wt], cmp[:, :Wt], 0.0, None,
                                    op0=mybir.AluOpType.is_lt)
            nc.vector.copy_predicated(acc[tt][:, :Wt], mbit[:, :Wt],
                                      negs[:, :Wt])
            # pack candidate index into the low 9 mantissa bits so one
            # max8+match_replace pass yields value AND index
            accu = acc[tt][:, :Wt].bitcast(U32)
            nc.vector.scalar_tensor_tensor(
                out=accu, in0=accu, scalar=maskhi, in1=cidx[:, :Wt],
                op0=mybir.AluOpType.bitwise_and,
                op1=mybir.AluOpType.bitwise_or)

            idx = tk.tile([128, TOPK], U32, tag="idx", name="idx")
            vals = acc[tt]
            for it in range(iters):
                mx = tk.tile([128, 8], F32, tag="mx", name="mx")
                nc.vector.max(out=mx, in_=vals[:, :Wt])
                nc.vector.tensor_scalar(idx[:, it * 8:(it + 1) * 8],
                                        mx.bitcast(U32), mask511, None,
                                        op0=mybir.AluOpType.bitwise_and)
                nc.vector.match_replace(out=vals[:, :Wt], in_to_replace=mx,
                                        in_values=vals[:, :Wt], imm_value=NEG)

            idx32 = tk.tile([128, TOPK], I32, tag="idx32", name="idx32")
            if nk < TOPK:
                nc.vector.memset(idx32[:, nk:], -1)
            nc.vector.tensor_copy(idx32[:, :nk], idx[:, :nk])
            rmp = work.tile([128, TOPK], F32, tag="rmp", name="rmp")
            nc.vector.tensor_scalar(rmp[:, :nk], c4f[:, :nk], pos_sb[tt], None,
                                    op0=mybir.AluOpType.add)
            rbit = work.tile([128, TOPK], U32, tag="rbit", name="rbit")
            nc.vector.tensor_scalar(rbit[:, :nk], rmp[:, :nk], 0.0, None,
                                    op0=mybir.AluOpType.is_lt)
            nc.vector.copy_predicated(idx32[:, :nk], rbit[:, :nk],
                                      neg1[:, :nk])
            nc.sync.dma_start(out=out_idx[tt * 128:(tt + 1) * 128, :],
                              in_=idx32)

        emit_all()
        emit_topk(0)
        emit_topk(1)

    nc.finalize()
    return nc


def _get(name, *args):
    key = (name, args)
    if key not in _cache:
        _cache[key] = _build_l1() if name == "l1" else _build_l2(*args)
    return _cache[key]


def kernel(hidden_states, qr, positions, W_fused, Wq, Wproj, ape, rms_weight,
           cos_sin_cache, _timing=None):
    hidden_states = np.asarray(hidden_states, np.float32)
    qr = np.asarray(qr, np.float32)
    positions = np.asarray(positions, np.int32)
    W_fused = np.asarray(W_fused, np.float32)
    Wq = np.asarray(Wq, np.float32)
    Wproj = np.asarray(Wproj, np.float32)
    ape = np.asarray(ape, np.float32)
    rms_weight = np.asarray(rms_weight, np.float32)
    cos_sin_cache = np.asarray(cos_sin_cache, np.float32)

    wcomb = np.ascontiguousarray(
        np.concatenate([W_fused.T, Wproj.T], axis=1))          # [7168, 576]
    wqT = np.ascontiguousarray(
        Wq.T.reshape(12, 128, H * D))                          # [kc, qr, m]

    cores = list(range(NC))
    trace = _timing is not None

    in1 = []
    for i in cores:
        rows = []
        for j in PAIRS[i]:
            lo = 128 * j - 4
            if lo < 0:
                blk = np.zeros((132, HID), np.float32)
                blk[4:] = hidden_states[0:128 * j + 128]
            else:
                blk = hidden_states[lo:128 * j + 128]
            rows.append(blk)
        hidden_halo = np.ascontiguousarray(np.concatenate(rows, axis=0))
        cs_rows = np.concatenate(
            [cos_sin_cache[32 * j:32 * j + 32] for j in PAIRS[i]], axis=0)
        hfl = np.array([NEG if j == 0 else 0.0 for j in PAIRS[i]], np.float32)
        in1.append({
            "hidden": hidden_halo, "wcomb": wcomb, "ape": ape,
            "rmsw": rms_weight, "cs_k": np.ascontiguousarray(cs_rows),
            "haloflag": hfl,
        })
    r1 = run_bass_kernel_spmd(_get("l1"), in1, cores, trace=trace,
                              trace_cores=cores if trace else None)

    k_full = np.zeros((C, D), np.float32)
    wts = {}
    for i in cores:
        kl = r1.results[i]["k_loc"]
        for s, j in enumerate(PAIRS[i]):
            k_full[32 * j:32 * j + 32] = kl[32 * s:32 * s + 32]
        wts[i] = r1.results[i]["wts_own"]

    # one SPMD program for all cores: tile A of core i covers tokens
    # 128i..128i+127 (causal width <= 256), tile B covers the mirror tile
    # (width <= 512); masked-NEG columns make the uniform widths correct.
    in2 = []
    for i in cores:
        sel = np.concatenate(
            [np.arange(128 * j, 128 * j + 128) for j in PAIRS[i]])
        in2.append({
            "qr_sh": np.ascontiguousarray(qr[sel]),
            "wqT": wqT,
            "cs_own": np.ascontiguousarray(cos_sin_cache[positions[sel]]),
            "k_full": k_full,
            "wts_own": wts[i],
            "posm3": (positions[sel] - 3).astype(np.float32),
        })
    r2 = run_bass_kernel_spmd(_get("l2", (256, 512)), in2, cores, trace=trace,
                              trace_cores=cores if trace else None)

    out = np.empty((T, TOPK), np.int32)
    for i in cores:
        oi = r2.results[i]["out_idx"]
        for s, j in enumerate(PAIRS[i]):
            out[128 * j:128 * j + 128] = oi[128 * s:128 * s + 128]

    if _timing is not None:
        _timing["l1"] = r1
        _timing["l2"] = r2
    return out
